# revision 2
# baseline (speedup 1.0000x reference)
"""HeteroGNN (GCN over authors + SAGE over papers) on 8 Trainium2 NeuronCores.

Strategy (graph/data parallel, per the sharding hint):
  - Papers and authors are sharded across the 8 cores by destination node.
  - Host preprocessing sorts each edge set by destination tile and bakes the
    GCN normalization / SAGE 1/cnt weights into per-tile one-hot "indicator"
    tensors Ind_T[e, d] (bf16).  On device, a message pass is:
        messages = dma_gather(table, src_ids)          # [128e, 128f] chunks
        psum[f, d] += messages_chunk.T @ ind_chunk     # PE segment-sum
  - GCN layer-0 output a1 is exchanged with a single AllGather collective;
    SAGE layer-1 then gathers a1 rows from the shared table.
  - The unused second GCN layer (a2) of the reference is dead code and skipped.

Everything flows in feature-major ("transposed") layouts so that no on-device
transposes are needed anywhere.
"""

import sys

sys.path.insert(0, "/opt/trn_rl_repo")

import numpy as np
import ml_dtypes

BF16 = ml_dtypes.bfloat16

# ---- problem constants (from spec) ----
N_CORES = 8
N_AUTHOR = 50000
N_PAPER = 25000
D = 128          # author feature dim == hidden dim
D_BERT = 768
FEAT = 12
OUT_D = 64
VOCAB = 31090

A_PC = N_AUTHOR // N_CORES      # 6250 authors per core
A_TILES = 49                    # ceil(6250/128)
A_PC_PAD = A_TILES * 128        # 6272
N_A_PAD = N_CORES * A_PC_PAD    # 50176 padded author table rows
P_PC = N_PAPER // N_CORES       # 3125 papers per core
P_TILES = 25
P_PC_PAD = P_TILES * 128        # 3200
SPLIT = 32768                   # int16 index limit for dma_gather

TRACE = False
LAST = {}

_CACHE = {}


def _pad_author_ids(v):
    """Map real author id -> padded table row (core-concat layout)."""
    return (v // A_PC) * A_PC_PAD + (v % A_PC)


def _wrap_idx(flat):
    """Lay out gather indices as [128, n/16] int16 (16-wrap, tiled x8)."""
    n = flat.shape[0]
    assert n % 16 == 0
    w = flat.reshape(n // 16, 16).T.astype(np.int16)  # [16, n/16]
    return np.tile(w, (8, 1))                          # [128, n/16]


def _prep_graph(src_pad, dst, w, n_dst_pc, n_tiles, grp):
    """Partition edges by destination shard/tile, split lo/hi by src row,
    pad to uniform per-tile chunk counts, and build per-core idx + indicator
    arrays.

    Returns (cfg, per_core_idx [128, TOT/16] int16, per_core_ind
    [n_tiles, 128, CMAX*128] bf16).
    """
    E = src_pad.shape[0]
    core = dst // n_dst_pc
    ld = dst - core * n_dst_pc
    tile = ld >> 7
    drel = ld & 127
    hi = (src_pad >= SPLIT).astype(np.int64)

    key = (core * n_tiles + tile) * 2 + hi
    order = np.argsort(key, kind="stable")
    key_s = key[order]
    src_s = src_pad[order]
    drel_s = drel[order]
    w_s = w[order]
    core_s = core[order]
    tile_s = tile[order]
    hi_s = hi[order]

    counts = np.bincount(key, minlength=N_CORES * n_tiles * 2)
    starts = np.concatenate([[0], np.cumsum(counts)[:-1]])
    rank = np.arange(E, dtype=np.int64) - starts[key_s]

    n_lo = counts[0::2].reshape(N_CORES, n_tiles)
    n_hi = counts[1::2].reshape(N_CORES, n_tiles)
    CLO = int(-(-n_lo.max() // 128))
    CHI = int(-(-n_hi.max() // 128))
    CLO = max(CLO, 1)
    CHI = max(CHI, 1)
    CMAX = CLO + CHI

    # groups of tiles share one lo-gather and one hi-gather call
    gsizes = []
    t0 = 0
    while t0 < n_tiles:
        gsizes.append(min(grp, n_tiles - t0))
        t0 += grp
    # pass-level offset (in idx slots) of each group's call region
    goff = np.concatenate([[0], np.cumsum([g * CMAX * 128 for g in gsizes])[:-1]])

    g_s = tile_s // grp
    ti_s = tile_s % grp
    gg = np.array(gsizes, dtype=np.int64)[g_s]
    base = goff[g_s] + np.where(
        hi_s == 0,
        ti_s * CLO * 128,
        gg * CLO * 128 + ti_s * CHI * 128,
    )
    pos = base + rank

    TOT = n_tiles * CMAX * 128
    idx_val = (src_s - hi_s * SPLIT).astype(np.int16)

    per_core_idx = []
    per_core_ind = []
    c_tile = np.where(hi_s == 0, rank >> 7, CLO + (rank >> 7))
    e_part = rank & 127
    col = c_tile * 128 + drel_s
    for c in range(N_CORES):
        m = core_s == c
        flat = np.zeros(TOT, dtype=np.int16)
        flat[pos[m]] = idx_val[m]
        per_core_idx.append(_wrap_idx(flat))
        ind = np.zeros((n_tiles, 128, CMAX * 128), dtype=BF16)
        ind[tile_s[m], e_part[m], col[m]] = w_s[m].astype(BF16)
        per_core_ind.append(ind)

    cfg = dict(CLO=CLO, CHI=CHI, CMAX=CMAX, gsizes=gsizes,
               goff=[int(x) for x in goff], n_tiles=n_tiles, TOT=TOT)
    return cfg, per_core_idx, per_core_ind


def _prep(inputs):
    """Host preprocessing: per-core in_maps + static builder config."""
    f32 = np.float32
    x_author = np.asarray(inputs["x_author"], f32)
    paper_tokens = np.asarray(inputs["paper_tokens"])
    paper_feat = np.asarray(inputs["paper_feat"], f32)
    edge_collab = np.asarray(inputs["edge_collab"], np.int64)
    writes_src = np.asarray(inputs["writes_src"], np.int64)
    writes_dst = np.asarray(inputs["writes_dst"], np.int64)

    # -- padded bf16 author table
    xa_pad = np.zeros((N_A_PAD, D), dtype=BF16)
    rows = _pad_author_ids(np.arange(N_AUTHOR))
    xa_pad[rows] = x_author.astype(BF16)

    emb = np.asarray(inputs["embed_table"], f32).astype(BF16)

    # -- GCN (collab -> authors): norm = dinv[src]*dinv[dst]; self loop dinv^2
    src_c, dst_c = edge_collab[0], edge_collab[1]
    deg = np.bincount(dst_c, minlength=N_AUTHOR).astype(f32) + 1.0
    dinv = 1.0 / np.sqrt(deg)
    vv = np.arange(N_AUTHOR, dtype=np.int64)
    g_src = np.concatenate([src_c, vv])
    g_dst = np.concatenate([dst_c, vv])
    g_w = np.concatenate([dinv[src_c] * dinv[dst_c], dinv * dinv]).astype(f32)
    cfg_c, idx_c, ind_c = _prep_graph(
        _pad_author_ids(g_src), g_dst, g_w, A_PC, A_TILES, grp=1)

    # -- SAGE (writes -> papers): weight 1/max(cnt,1)
    cnt = np.bincount(writes_dst, minlength=N_PAPER).astype(f32)
    s_w = (1.0 / np.maximum(cnt, 1.0))[writes_dst].astype(f32)
    cfg_w, idx_w, ind_w = _prep_graph(
        _pad_author_ids(writes_src), writes_dst, s_w, P_PC, P_TILES, grp=1)

    # -- weights (host-reshaped to device layouts, bf16)
    def brow(name, n):
        return np.asarray(inputs[name], f32).reshape(1, n).astype(BF16)

    Wp = np.asarray(inputs["Wp"], f32)  # [768, 768] [k, f]
    wp_sb = Wp.reshape(6, 128, D_BERT).transpose(1, 0, 2).reshape(128, 6 * D_BERT)
    Wr0 = np.asarray(inputs["sage_Wr0"], f32)  # [780, 128]
    wr0_sb = Wr0[:768].reshape(6, 128, 128).transpose(1, 0, 2).reshape(128, 6 * 128)
    wr0f = Wr0[768:780]  # [12, 128]

    consts = dict(
        wp=wp_sb.astype(BF16),
        bp=brow("bp", D_BERT),
        w0=np.asarray(inputs["gcn_W0"], f32).astype(BF16),
        b0=brow("gcn_b0", D),
        wl0=np.asarray(inputs["sage_Wl0"], f32).astype(BF16),
        wr0=wr0_sb.astype(BF16),
        wr0f=wr0f.astype(BF16),
        sb0=brow("sage_b0", D),
        wl1=np.asarray(inputs["sage_Wl1"], f32).astype(BF16),
        wr1=np.asarray(inputs["sage_Wr1"], f32).astype(BF16),
        sb1=brow("sage_b1", D),
        linw=np.asarray(inputs["lin_W"], f32).astype(BF16),
        linb=brow("lin_b", OUT_D),
        ones=np.ones((1, 512), dtype=BF16),
    )

    in_maps = []
    for c in range(N_CORES):
        p0, p1 = c * P_PC, (c + 1) * P_PC
        cls = np.zeros(P_PC_PAD, dtype=np.int64)
        cls[:P_PC] = paper_tokens[p0:p1, 0]
        featT = np.zeros((FEAT, P_PC_PAD), dtype=BF16)
        featT[:, :P_PC] = paper_feat[p0:p1].T.astype(BF16)
        m = dict(
            emb=emb,
            clsidx=_wrap_idx(cls),
            featT=featT,
            xa=xa_pad,
            idxc=idx_c[c],
            indc=ind_c[c],
            idxw=idx_w[c],
            indw=ind_w[c],
            **consts,
        )
        in_maps.append(m)

    return in_maps, (cfg_c, cfg_w)


def _build(cfg_c, cfg_w):
    """Build the SPMD Bass program (shared by all 8 cores)."""
    import concourse.bacc as bacc
    import concourse.bass as bass
    import concourse.mybir as mybir
    from concourse.tile import TileContext

    dt = mybir.dt
    AF = mybir.ActivationFunctionType
    ALU = mybir.AluOpType

    nc = bacc.Bacc("TRN2", target_bir_lowering=False, debug=False,
                   num_devices=N_CORES)

    # ---- I/O declarations
    def din(name, shape, dtype=dt.bfloat16):
        return nc.dram_tensor(name, list(shape), dtype, kind="ExternalInput").ap()

    emb = din("emb", (VOCAB, D_BERT))
    clsidx = din("clsidx", (128, P_PC_PAD // 16), dt.int16)
    featT = din("featT", (FEAT, P_PC_PAD))
    xa = din("xa", (N_A_PAD, D))
    wp = din("wp", (128, 6 * D_BERT))
    bp = din("bp", (1, D_BERT))
    w0 = din("w0", (D, D))
    b0 = din("b0", (1, D))
    wl0 = din("wl0", (D, D))
    wr0 = din("wr0", (128, 6 * 128))
    wr0f = din("wr0f", (FEAT, D))
    sb0 = din("sb0", (1, D))
    wl1 = din("wl1", (D, D))
    wr1 = din("wr1", (D, D))
    sb1 = din("sb1", (1, D))
    linw = din("linw", (D, OUT_D))
    linb = din("linb", (1, OUT_D))
    ones = din("ones", (1, 512))
    idxc = din("idxc", (128, cfg_c["TOT"] // 16), dt.int16)
    indc = din("indc", (A_TILES, 128, cfg_c["CMAX"] * 128))
    idxw = din("idxw", (128, cfg_w["TOT"] // 16), dt.int16)
    indw = din("indw", (P_TILES, 128, cfg_w["CMAX"] * 128))
    out = nc.dram_tensor("out", [P_PC_PAD, OUT_D], dt.float32,
                         kind="ExternalOutput").ap()

    MSG_ELEMS = max(cfg_c["CMAX"], cfg_w["CMAX"]) * 128

    with TileContext(nc) as tc:
        with (
            tc.tile_pool(name="const", bufs=1) as constp,
            tc.tile_pool(name="big", bufs=1) as bigp,
            tc.tile_pool(name="msg", bufs=3) as msgp,
            tc.tile_pool(name="ind", bufs=2) as indp,
            tc.tile_pool(name="idx", bufs=3) as idxp,
            tc.tile_pool(name="work", bufs=3) as workp,
            tc.tile_pool(name="psum", bufs=2, space="PSUM") as psump,
            tc.tile_pool(name="dram", bufs=1, space="DRAM") as dramp,
        ):
            # ---- constants to SBUF
            def load_const(ap_dram, name):
                t = constp.tile(list(ap_dram.shape), ap_dram.dtype, name=name)
                nc.sync.dma_start(out=t, in_=ap_dram)
                return t

            wp_sb = load_const(wp, "wp_sb")
            bp_sb = load_const(bp, "bp_sb")
            w0_sb = load_const(w0, "w0_sb")
            b0_sb = load_const(b0, "b0_sb")
            wl0_sb = load_const(wl0, "wl0_sb")
            wr0_sb = load_const(wr0, "wr0_sb")
            wr0f_sb = load_const(wr0f, "wr0f_sb")
            sb0_sb = load_const(sb0, "sb0_sb")
            wl1_sb = load_const(wl1, "wl1_sb")
            wr1_sb = load_const(wr1, "wr1_sb")
            sb1_sb = load_const(sb1, "sb1_sb")
            linw_sb = load_const(linw, "linw_sb")
            linb_sb = load_const(linb, "linb_sb")
            ones_sb = load_const(ones, "ones_sb")
            clsidx_sb = load_const(clsidx, "clsidx_sb")
            featT_sb = load_const(featT, "featT_sb")

            poolerT = bigp.tile([128, 6 * P_PC_PAD], dt.bfloat16, name="poolerT")
            p1T = bigp.tile([128, P_PC_PAD], dt.bfloat16, name="p1T")

            a1_shard = dramp.tile([A_PC_PAD, D], dt.bfloat16, name="a1_shard")
            a1_full = dramp.tile([N_A_PAD, D], dt.bfloat16,
                                 addr_space="Shared", name="a1_full")

            # =========== pooler: poolerT[f, p] = tanh(Wp.T-chunks @ clsT + bp)
            clsT = bigp.tile([128, P_TILES * D_BERT], dt.bfloat16,
                             name="clsT")
            for i in range(P_TILES):
                c3 = clsT[:, i * D_BERT:(i + 1) * D_BERT].rearrange(
                    "p (c e) -> p c e", e=128)
                nc.gpsimd.dma_gather(
                    c3, emb, clsidx_sb[:, i * 8:(i + 1) * 8], 128, 128, D_BERT,
                    transpose=True,
                )
            for ft in range(6):
                for t in range(P_TILES):
                    ps = psump.tile([128, 128], dt.float32, tag="pool",
                                    name="ps_pool")
                    for c in range(6):
                        nc.tensor.matmul(
                            ps,
                            lhsT=wp_sb[:, c * D_BERT + ft * 128:
                                       c * D_BERT + ft * 128 + 128],
                            rhs=clsT[:, t * D_BERT + c * 128:
                                     t * D_BERT + (c + 1) * 128],
                            start=(c == 0), stop=False,
                        )
                    nc.tensor.matmul(ps, lhsT=bp_sb[:, ft * 128:(ft + 1) * 128],
                                     rhs=ones_sb[:, :128], start=False,
                                     stop=True)
                    nc.scalar.activation(
                        poolerT[:, ft * P_PC_PAD + t * 128:
                                ft * P_PC_PAD + (t + 1) * 128],
                        ps, AF.Tanh)

            # =========== shared message-pass emitter
            def message_pass(cfg, idx_dram, ind_dram, table_lo, table_hi,
                             n_tiles, consume, pname):
                CLO, CHI, CMAX = cfg["CLO"], cfg["CHI"], cfg["CMAX"]
                for t in range(n_tiles):
                    coff = t * CMAX * 128
                    idxt = idxp.tile([128, CMAX * 8], dt.int16, tag="idx",
                                     name=f"idxt_{pname}")
                    nc.sync.dma_start(
                        out=idxt,
                        in_=idx_dram[:, coff // 16: coff // 16 + CMAX * 8])
                    msg = msgp.tile([128, MSG_ELEMS], dt.bfloat16, tag="msg",
                                    name=f"msg_{pname}")
                    m3 = msg[:, : CMAX * 128].rearrange("p (c e) -> p c e",
                                                        e=128)
                    c0 = 0
                    while c0 < CLO:
                        nch = min(8, CLO - c0)
                        nc.gpsimd.dma_gather(
                            m3[:, c0:c0 + nch, :], table_lo,
                            idxt[:, c0 * 8:(c0 + nch) * 8],
                            nch * 128, nch * 128, D)
                        c0 += nch
                    while c0 < CMAX:
                        nch = min(8, CMAX - c0)
                        nc.gpsimd.dma_gather(
                            m3[:, c0:c0 + nch, :], table_hi,
                            idxt[:, c0 * 8:(c0 + nch) * 8],
                            nch * 128, nch * 128, D)
                        c0 += nch
                    ind = indp.tile([128, CMAX * 128], dt.bfloat16,
                                    tag="ind", name=f"ind_{pname}")
                    nc.sync.dma_start(out=ind, in_=ind_dram[t])
                    agg = psump.tile([128, 128], dt.float32, tag="agg",
                                     name=f"agg_{pname}")
                    for c in range(CMAX):
                        nc.tensor.matmul(
                            agg,
                            lhsT=m3[:, c, :],
                            rhs=ind[:, c * 128:(c + 1) * 128],
                            start=(c == 0), stop=(c == CMAX - 1),
                        )
                    aggsb = workp.tile([128, 128], dt.bfloat16, tag="aggsb",
                                       name=f"aggsb_{pname}")
                    nc.vector.tensor_copy(aggsb, agg)
                    consume(t, aggsb)

            # =========== GCN layer 0 -> a1 shard, then AllGather
            def gcn_consume(t, aggsb):
                lp = psump.tile([128, 128], dt.float32, tag="layer",
                                name="lp_gcn")
                nc.tensor.matmul(lp, lhsT=aggsb, rhs=w0_sb,
                                 start=True, stop=False)
                nc.tensor.matmul(lp, lhsT=ones_sb[:, :128], rhs=b0_sb,
                                 start=False, stop=True)
                a1sb = workp.tile([128, 128], dt.bfloat16, tag="a1sb",
                                  name="a1sb")
                nc.scalar.activation(a1sb, lp, AF.Relu)
                nc.sync.dma_start(out=a1_shard[t * 128:(t + 1) * 128, :],
                                  in_=a1sb)

            message_pass(cfg_c, idxc, indc, xa[0:SPLIT, :], xa[SPLIT:N_A_PAD, :],
                         A_TILES, gcn_consume, "gcn")

            nc.gpsimd.collective_compute(
                "AllGather", ALU.bypass,
                replica_groups=[list(range(N_CORES))],
                ins=[a1_shard.opt()], outs=[a1_full.opt()],
            )

            # =========== SAGE layer 0 -> p1T[h, d]
            def sage0_consume(t, meansb):
                pp = psump.tile([128, 128], dt.float32, tag="layer",
                                name="pp_s0")
                nc.tensor.matmul(pp, lhsT=wl0_sb, rhs=meansb,
                                 start=True, stop=False)
                for c in range(6):
                    nc.tensor.matmul(
                        pp,
                        lhsT=wr0_sb[:, c * 128:(c + 1) * 128],
                        rhs=poolerT[:, c * P_PC_PAD + t * 128:
                                    c * P_PC_PAD + (t + 1) * 128],
                        start=False, stop=False)
                nc.tensor.matmul(pp, lhsT=wr0f_sb,
                                 rhs=featT_sb[:, t * 128:(t + 1) * 128],
                                 start=False, stop=False)
                nc.tensor.matmul(pp, lhsT=sb0_sb, rhs=ones_sb[:, :128],
                                 start=False, stop=True)
                nc.scalar.activation(p1T[:, t * 128:(t + 1) * 128], pp, AF.Relu)

            message_pass(cfg_w, idxw, indw, xa[0:SPLIT, :], xa[SPLIT:N_A_PAD, :],
                         P_TILES, sage0_consume, "s0")

            # =========== SAGE layer 1 + head -> out
            def sage1_consume(t, meansb):
                pp = psump.tile([128, 128], dt.float32, tag="layer",
                                name="pp_s1")
                nc.tensor.matmul(pp, lhsT=wl1_sb, rhs=meansb,
                                 start=True, stop=False)
                nc.tensor.matmul(pp, lhsT=wr1_sb,
                                 rhs=p1T[:, t * 128:(t + 1) * 128],
                                 start=False, stop=False)
                nc.tensor.matmul(pp, lhsT=sb1_sb, rhs=ones_sb[:, :128],
                                 start=False, stop=True)
                p2sb = workp.tile([128, 128], dt.bfloat16, tag="p2sb",
                                  name="p2sb")
                nc.scalar.activation(p2sb, pp, AF.Relu)
                hp = psump.tile([128, OUT_D], dt.float32, tag="head",
                                name="hp")
                nc.tensor.matmul(hp, lhsT=p2sb, rhs=linw_sb,
                                 start=True, stop=False)
                nc.tensor.matmul(hp, lhsT=ones_sb[:, :128], rhs=linb_sb,
                                 start=False, stop=True)
                outsb = workp.tile([128, OUT_D], dt.float32, tag="outsb",
                                   name="outsb")
                nc.vector.tensor_copy(outsb, hp)
                nc.sync.dma_start(out=out[t * 128:(t + 1) * 128, :], in_=outsb)

            message_pass(cfg_w, idxw, indw, a1_full[0:SPLIT, :],
                         a1_full[SPLIT:N_A_PAD, :],
                         P_TILES, sage1_consume, "s1")

    nc.compile()
    return nc


def kernel(**inputs):
    from concourse import bass_utils

    in_maps, (cfg_c, cfg_w) = _prep(inputs)

    key = (cfg_c["CLO"], cfg_c["CHI"], cfg_w["CLO"], cfg_w["CHI"])
    if key in _CACHE:
        nc = _CACHE[key]
    else:
        nc = _build(cfg_c, cfg_w)
        _CACHE[key] = nc

    kw = {}
    if TRACE:
        import os
        os.makedirs("/tmp/neff_trace", exist_ok=True)
        kw = dict(tmpdir="/tmp/neff_trace")
    res = bass_utils.run_bass_kernel_spmd(
        nc, in_maps, core_ids=list(range(N_CORES)), trace=TRACE, **kw)
    LAST["exec_time_ns"] = res.exec_time_ns
    LAST["results"] = res

    pieces = [res.results[c]["out"][:P_PC] for c in range(N_CORES)]
    return np.concatenate(pieces, axis=0).astype(np.float32)



# revision 11
# speedup vs baseline: 1.3142x; 1.3142x over previous
"""HeteroGNN (GCN over authors + SAGE over papers) on 8 Trainium2 NeuronCores.

v2 design (from trace analysis of the v1 baseline, which was ~3.1 ms and
99% GPSIMD-bound on dma_gather descriptor generation):
  - dma_gather calls round-robin over 4 SWDGE queues (queue_num selects the
    Q7 core pair that generates descriptors -> ~3x parallel desc-gen).
  - sage0+sage1 share ONE gather per writes-edge: 512B rows from a combined
    [xa | a1] table, AllGathered in 4 tile-sliced collectives so the
    collective overlaps the GCN tail; sliced tables also fit int16 indices
    without a lo/hi split.
  - Segment-sum one-hots are PURE 0/1: GCN's per-edge norm is folded into a
    pre-scaled gather table (dinv[src]*xa) plus a per-tile dst post-scale;
    SAGE's 1/cnt is a per-tile dst post-scale.  Pure one-hots are
    host-staged in fp8 and DMA'd via HWDGE (nc.sync) - PE accepts
    bf16 lhsT x fp8 rhs exactly (verified on HW).
  - GCN self-loops: host-staged self term added during the psum->SBUF copy.
  - Pooler GEMM restructured weight-stationary with 512-wide streams; its
    output is pre-multiplied by sage_Wr0 into xpwr so the SAGE loop only
    adds a slice.
"""

import sys

sys.path.insert(0, "/opt/trn_rl_repo")

import numpy as np
import ml_dtypes

BF16 = ml_dtypes.bfloat16
FP8 = ml_dtypes.float8_e4m3

# ---- problem constants ----
N_CORES = 8
N_AUTHOR = 50000
N_PAPER = 25000
D = 128
D_BERT = 768
FEAT = 12
OUT_D = 64
VOCAB = 31090

A_PC = N_AUTHOR // N_CORES      # 6250
A_TILES = 49
A_PC_PAD = A_TILES * 128        # 6272
N_A_PAD = N_CORES * A_PC_PAD    # 50176
P_PC = N_PAPER // N_CORES       # 3125
P_TILES = 25
P_PC_PAD = P_TILES * 128        # 3200
SPLIT = 32768                   # int16 idx limit for the GCN table
CALL_CHUNKS = 8                 # max 1024 rows per dma_gather call
N_QUEUES = 1

N_SLICES = 4                    # AllGather slices (author tile ranges)
SLICE_TILES = [13, 13, 13, 10]
SLICE_T0 = np.concatenate([[0], np.cumsum(SLICE_TILES)]).astype(np.int64)

GCN_GROUP = 4                   # dst tiles whose agg psums are open together
SAGE_GROUP = 2

BLK = 512                       # pooler block (psum + cls-gather granularity)

TRACE = False
LAST = {}
_CACHE = {}


def _wrap_idx(flat):
    n = flat.shape[0]
    assert n % 16 == 0
    w = flat.reshape(n // 16, 16).T.astype(np.int16)
    return np.tile(w, (8, 1))


def _pad_author_ids(v):
    return (v // A_PC) * A_PC_PAD + (v % A_PC)


def _common_layout(counts_list, n_tiles, n_tabs, group):
    """SPMD-common chunk layout: per (tile, table) the max chunk count over
    cores; runs ordered [group: [table: [tile: run]]]; calls merge adjacent
    same-table chunks up to CALL_CHUNKS."""
    nch = np.zeros((n_tiles, n_tabs), dtype=np.int64)
    for counts in counts_list:
        nch = np.maximum(nch, -(-counts // 128))

    runs = []                      # (tile, table, chunk0, n)
    tile_runs = [[] for _ in range(n_tiles)]
    cursor = 0
    for g0 in range(0, n_tiles, group):
        g1 = min(g0 + group, n_tiles)
        for tb in range(n_tabs):
            for t in range(g0, g1):
                n = int(nch[t, tb])
                if n == 0:
                    continue
                runs.append((t, tb, cursor, n))
                tile_runs[t].append((cursor, n))
                cursor += n

    calls = []
    for t, tb, c0, n in runs:
        if calls and calls[-1][0] == tb and \
                calls[-1][1] + calls[-1][2] == c0 and \
                calls[-1][2] + n <= CALL_CHUNKS:
            calls[-1] = (tb, calls[-1][1], calls[-1][2] + n)
        else:
            while n > CALL_CHUNKS:
                calls.append((tb, c0, CALL_CHUNKS))
                c0 += CALL_CHUNKS
                n -= CALL_CHUNKS
            if n:
                calls.append((tb, c0, n))
    # split any call that grew past the cap via merging
    final = []
    for tb, c0, n in calls:
        while n > CALL_CHUNKS:
            final.append((tb, c0, CALL_CHUNKS))
            c0 += CALL_CHUNKS
            n -= CALL_CHUNKS
        if n:
            final.append((tb, c0, n))
    return dict(nch=nch, runs=runs, tile_runs=tile_runs, calls=final,
                total_chunks=cursor)


def _fill_core(layout, srow, tab, tile, drel):
    """Lay this core's edges into the common layout.  Returns
    (idx_flat [TOT*128] int64, oh [TOT*128, 128] fp8)."""
    tot = layout["total_chunks"]
    idx_flat = np.zeros(tot * 128, dtype=np.int64)
    dr_all = np.full(tot * 128, -1, dtype=np.int64)
    for t, tb, c0, n in layout["runs"]:
        m = (tile == t) & (tab == tb)
        e = np.nonzero(m)[0]
        ne = len(e)
        assert ne <= n * 128
        idx_flat[c0 * 128: c0 * 128 + ne] = srow[e]
        dr_all[c0 * 128: c0 * 128 + ne] = drel[e]
    oh = np.zeros((tot * 128, 128), dtype=FP8)
    rr = np.nonzero(dr_all >= 0)[0]
    oh[rr, dr_all[rr]] = np.float32(1.0)
    return idx_flat, oh


def _prep(inputs):
    f32 = np.float32
    x_author = np.asarray(inputs["x_author"], f32)
    paper_tokens = np.asarray(inputs["paper_tokens"])
    paper_feat = np.asarray(inputs["paper_feat"], f32)
    edge_collab = np.asarray(inputs["edge_collab"], np.int64)
    writes_src = np.asarray(inputs["writes_src"], np.int64)
    writes_dst = np.asarray(inputs["writes_dst"], np.int64)

    src_c, dst_c = edge_collab[0], edge_collab[1]
    deg = np.bincount(dst_c, minlength=N_AUTHOR).astype(f32) + 1.0
    dinv = 1.0 / np.sqrt(deg)

    rows = _pad_author_ids(np.arange(N_AUTHOR))
    xa_gcn = np.zeros((N_A_PAD, D), dtype=BF16)       # dinv[v] * xa[v]
    xa_gcn[rows] = (x_author * dinv[:, None]).astype(BF16)
    cnt = np.bincount(writes_dst, minlength=N_PAPER).astype(f32)
    invcnt = 1.0 / np.maximum(cnt, 1.0)

    emb = np.asarray(inputs["embed_table"], f32).astype(BF16)

    # ---- GCN edge partition (per core, common layout)
    src_c_pad = _pad_author_ids(src_c)
    gcn_cores = []
    gcn_counts = []
    for c in range(N_CORES):
        m = (dst_c >= c * A_PC) & (dst_c < (c + 1) * A_PC)
        s = src_c_pad[m]
        dloc = dst_c[m] - c * A_PC
        tab = (s >= SPLIT).astype(np.int64)
        srow = np.where(tab == 0, s, s - SPLIT)
        tile = dloc >> 7
        drel = dloc & 127
        gcn_cores.append((srow, tab, tile, drel))
        cnts = np.zeros((A_TILES, 2), dtype=np.int64)
        np.add.at(cnts, (tile, tab), 1)
        gcn_counts.append(cnts)
    gcn_layout = _common_layout(gcn_counts, A_TILES, 2, GCN_GROUP)

    # ---- SAGE edge partition: table id = AllGather stripe
    ws_pad = _pad_author_ids(writes_src)
    ws_block = ws_pad // A_PC_PAD
    ws_local = ws_pad % A_PC_PAD
    ws_tile = ws_local >> 7
    ws_stripe = np.searchsorted(SLICE_T0[1:], ws_tile, side="right")
    stripe_rows = np.array([n * 128 for n in SLICE_TILES])
    off_rows = SLICE_T0[:-1] * 128
    ws_row = ws_block * stripe_rows[ws_stripe] + (ws_local - off_rows[ws_stripe])
    sage_cores = []
    sage_counts = []
    for c in range(N_CORES):
        m = (writes_dst >= c * P_PC) & (writes_dst < (c + 1) * P_PC)
        dloc = writes_dst[m] - c * P_PC
        tile = dloc >> 7
        drel = dloc & 127
        sage_cores.append((ws_row[m], ws_stripe[m], tile, drel))
        cnts = np.zeros((P_TILES, N_SLICES), dtype=np.int64)
        np.add.at(cnts, (tile, ws_stripe[m]), 1)
        sage_counts.append(cnts)
    sage_layout = _common_layout(sage_counts, P_TILES, N_SLICES, SAGE_GROUP)

    # ---- weights
    def brow(name, n):
        return np.asarray(inputs[name], f32).reshape(1, n).astype(BF16)

    Wp = np.asarray(inputs["Wp"], f32)
    wp_sb = Wp.reshape(6, 128, D_BERT).transpose(1, 0, 2).reshape(128, 6 * D_BERT)
    Wr0 = np.asarray(inputs["sage_Wr0"], f32)
    wr0_sb = Wr0[:768].reshape(6, 128, 128).transpose(1, 0, 2).reshape(128, 6 * 128)
    wr0f = Wr0[768:780]

    consts = dict(
        wp=wp_sb.astype(BF16),
        bp=brow("bp", D_BERT),
        w0=np.asarray(inputs["gcn_W0"], f32).astype(BF16),
        b0=brow("gcn_b0", D),
        wl0=np.asarray(inputs["sage_Wl0"], f32).astype(BF16),
        wr0=wr0_sb.astype(BF16),
        wr0f=wr0f.astype(BF16),
        sb0=brow("sage_b0", D),
        wl1=np.asarray(inputs["sage_Wl1"], f32).astype(BF16),
        wr1=np.asarray(inputs["sage_Wr1"], f32).astype(BF16),
        sb1=brow("sage_b1", D),
        linw=np.asarray(inputs["lin_W"], f32).astype(BF16),
        linb=brow("lin_b", OUT_D),
        ones=np.ones((1, 512), dtype=BF16),
        iden=np.eye(128, dtype=BF16),
    )

    in_maps = []
    for c in range(N_CORES):
        a0 = c * A_PC
        dv = np.zeros(A_PC_PAD, f32)
        dv[:A_PC] = dinv[a0:a0 + A_PC]
        dinvrep = np.tile(dv[None, :].astype(BF16), (128, 1))
        selfT = np.zeros((D, A_PC_PAD), dtype=BF16)
        selfT[:, :A_PC] = (x_author[a0:a0 + A_PC] *
                           (dv[:A_PC] ** 2)[:, None]).T.astype(BF16)
        cmb_shard = np.zeros((A_PC_PAD, 2 * D), dtype=BF16)
        cmb_shard[:A_PC, :D] = x_author[a0:a0 + A_PC].astype(BF16)

        p0 = c * P_PC
        iv = np.zeros(P_PC_PAD, f32)
        iv[:P_PC] = invcnt[p0:p0 + P_PC]
        invcntrep = np.tile(iv[None, :].astype(BF16), (128, 1))

        cls = np.zeros(P_PC_PAD, dtype=np.int64)
        cls[:P_PC] = paper_tokens[p0:p0 + P_PC, 0]
        featT = np.zeros((FEAT, P_PC_PAD), dtype=BF16)
        featT[:, :P_PC] = paper_feat[p0:p0 + P_PC].T.astype(BF16)

        gidx, goh = _fill_core(gcn_layout, *gcn_cores[c])
        sidx, soh = _fill_core(sage_layout, *sage_cores[c])

        m = dict(
            emb=emb,
            xa_gcn=xa_gcn,
            cmb_shard=cmb_shard,
            clsidx=_wrap_idx(cls),
            featT=featT,
            dinvrep=dinvrep,
            selfT=selfT,
            invcntrep=invcntrep,
            gcn_idx=_wrap_idx(gidx),
            gcn_oh=goh,
            sage_idx=_wrap_idx(sidx),
            sage_oh=soh,
            **consts,
        )
        in_maps.append(m)

    return in_maps, (gcn_layout, sage_layout)


def _build(gcn_layout, sage_layout):
    import concourse.bacc as bacc
    import concourse.mybir as mybir
    from concourse.tile import TileContext

    dt = mybir.dt
    AF = mybir.ActivationFunctionType
    ALU = mybir.AluOpType

    GCN_CH = gcn_layout["total_chunks"]
    SAGE_CH = sage_layout["total_chunks"]

    nc = bacc.Bacc("TRN2", target_bir_lowering=False, debug=False,
                   num_devices=N_CORES, num_swdge_queues=N_QUEUES)

    def din(name, shape, dtype=dt.bfloat16):
        return nc.dram_tensor(name, list(shape), dtype, kind="ExternalInput").ap()

    emb = din("emb", (VOCAB, D_BERT))
    xa_gcn = din("xa_gcn", (N_A_PAD, D))
    cmb_shard = din("cmb_shard", (A_PC_PAD, 2 * D))
    clsidx = din("clsidx", (128, P_PC_PAD // 16), dt.int16)
    featT = din("featT", (FEAT, P_PC_PAD))
    dinvrep = din("dinvrep", (128, A_PC_PAD))
    selfT = din("selfT", (D, A_PC_PAD))
    invcntrep = din("invcntrep", (128, P_PC_PAD))
    gcn_idx = din("gcn_idx", (128, GCN_CH * 8), dt.int16)
    gcn_oh = din("gcn_oh", (GCN_CH * 128, 128), dt.float8e4)
    sage_idx = din("sage_idx", (128, SAGE_CH * 8), dt.int16)
    sage_oh = din("sage_oh", (SAGE_CH * 128, 128), dt.float8e4)
    wp = din("wp", (128, 6 * D_BERT))
    bp = din("bp", (1, D_BERT))
    w0 = din("w0", (D, D))
    b0 = din("b0", (1, D))
    wl0 = din("wl0", (D, D))
    wr0 = din("wr0", (128, 6 * 128))
    wr0f = din("wr0f", (FEAT, D))
    sb0 = din("sb0", (1, D))
    wl1 = din("wl1", (D, D))
    wr1 = din("wr1", (D, D))
    sb1 = din("sb1", (1, D))
    linw = din("linw", (D, OUT_D))
    linb = din("linb", (1, OUT_D))
    ones = din("ones", (1, 512))
    iden = din("iden", (128, 128))
    out = nc.dram_tensor("out", [P_PC_PAD, OUT_D], dt.float32,
                         kind="ExternalOutput").ap()

    qrr = [0]

    def next_q():
        q = qrr[0]
        qrr[0] = (q + 1) % N_QUEUES
        return q

    with TileContext(nc) as tc:
        with (
            tc.tile_pool(name="const", bufs=1) as constp,
            tc.tile_pool(name="sticky", bufs=1) as stickyp,
            tc.tile_pool(name="cls", bufs=2) as clsp,
            tc.tile_pool(name="msg", bufs=10) as msgp,
            tc.tile_pool(name="ohp", bufs=10) as ohp,
            tc.tile_pool(name="work", bufs=4) as workp,
            tc.tile_pool(name="psum", bufs=2, space="PSUM") as psump,
            tc.tile_pool(name="psagg", bufs=4, space="PSUM") as psaggp,
            tc.tile_pool(name="pspool", bufs=1, space="PSUM") as pspoolp,
            tc.tile_pool(name="dram", bufs=1, space="DRAM") as dramp,
        ):
            def load_const(ap_dram, name):
                t = constp.tile(list(ap_dram.shape), ap_dram.dtype, name=name)
                nc.sync.dma_start(out=t, in_=ap_dram)
                return t

            w0_sb = load_const(w0, "w0_sb")
            b0_sb = load_const(b0, "b0_sb")
            wl0_sb = load_const(wl0, "wl0_sb")
            wr0_sb = load_const(wr0, "wr0_sb")
            wr0f_sb = load_const(wr0f, "wr0f_sb")
            sb0_sb = load_const(sb0, "sb0_sb")
            wl1_sb = load_const(wl1, "wl1_sb")
            wr1_sb = load_const(wr1, "wr1_sb")
            sb1_sb = load_const(sb1, "sb1_sb")
            linw_sb = load_const(linw, "linw_sb")
            linb_sb = load_const(linb, "linb_sb")
            ones_sb = load_const(ones, "ones_sb")
            iden_sb = load_const(iden, "iden_sb")
            wp_sb = load_const(wp, "wp_sb")
            bp_sb = load_const(bp, "bp_sb")
            featT_sb = load_const(featT, "featT_sb")
            clsidx_sb = load_const(clsidx, "clsidx_sb")
            dinvrep_sb = load_const(dinvrep, "dinvrep_sb")
            selfT_sb = load_const(selfT, "selfT_sb")
            invcntrep_sb = load_const(invcntrep, "invcntrep_sb")
            gcn_idx_sb = load_const(gcn_idx, "gcn_idx_sb")
            sage_idx_sb = load_const(sage_idx, "sage_idx_sb")

            xpwr = stickyp.tile([128, P_PC_PAD], dt.bfloat16, name="xpwr")
            p1T = stickyp.tile([128, P_PC_PAD], dt.bfloat16, name="p1T")

            cmb_full = [
                dramp.tile([N_CORES * SLICE_TILES[s] * 128, 2 * D],
                           dt.bfloat16, addr_space="Shared",
                           name=f"cmb_full{s}")
                for s in range(N_SLICES)
            ]
            # device-owned shard buffer (Tile tracks deps on pool tiles);
            # xa half copied from the host-staged input, a1 half written
            # by the GCN consume step.
            cmb_dev = dramp.tile([A_PC_PAD, 2 * D], dt.bfloat16,
                                 name="cmb_dev")
            nc.sync.dma_start(out=cmb_dev[:, 0:D], in_=cmb_shard[:, 0:D])

            # ================= pass machinery =================
            def make_pass(layout, idx_sb, oh_dram, tables, elem, pname,
                          n_tiles, group, n_psums, chunk_mm, consume):
                calls = layout["calls"]
                tile_runs = layout["tile_runs"]
                chunk_loc = {}
                for k, (tb, c0, nchk) in enumerate(calls):
                    for i in range(nchk):
                        chunk_loc[c0 + i] = (k, i)
                msg_tiles = {}
                oh_tiles = {}

                def ensure_call(k):
                    tb, c0, nchk = calls[k]
                    mt = msgp.tile([128, CALL_CHUNKS * elem], dt.bfloat16,
                                   tag="msg", name=f"msg_{pname}_{k}")
                    nc.gpsimd.dma_gather(
                        mt[:, :nchk * elem].rearrange(
                            "p (c e) -> p c e", e=elem),
                        tables[tb], idx_sb[:, c0 * 8:(c0 + nchk) * 8],
                        nchk * 128, nchk * 128, elem, queue_num=next_q())
                    ot = ohp.tile([128, CALL_CHUNKS * 128], dt.float8e4,
                                  tag="oh", name=f"oh_{pname}_{k}")
                    nc.sync.dma_start(
                        out=ot[:, :nchk * 128].rearrange(
                            "p (c e) -> p c e", e=128),
                        in_=oh_dram[c0 * 128:(c0 + nchk) * 128, :].rearrange(
                            "(c p) e -> p c e", p=128))
                    msg_tiles[k] = mt
                    oh_tiles[k] = ot

                issued = [0]

                def issue_upto(k):
                    while issued[0] <= min(k + 2, len(calls) - 1):
                        ensure_call(issued[0])
                        issued[0] += 1

                g0 = 0
                while g0 < n_tiles:
                    g1 = min(g0 + group, n_tiles)
                    last_chunk = max(
                        r0 + nr - 1
                        for t in range(g0, g1) for (r0, nr) in tile_runs[t])
                    issue_upto(chunk_loc[last_chunk][0])
                    for t in range(g0, g1):
                        pss = [psaggp.tile([128, 128], dt.float32, tag="agg",
                                           name=f"agg_{pname}_{t}_{j}")
                               for j in range(n_psums)]
                        runs = tile_runs[t]
                        n_run_chunks = sum(nr for _, nr in runs)
                        ci = 0
                        for (r0, nr) in runs:
                            for i in range(nr):
                                k, off = chunk_loc[r0 + i]
                                chunk_mm(pss, msg_tiles[k], oh_tiles[k], off,
                                         first=(ci == 0),
                                         last=(ci == n_run_chunks - 1))
                                ci += 1
                        consume(t, pss)
                    g0 = g1

            # ================= phase 1: GCN =================
            def gcn_chunk_mm(pss, mt, ot, off, first, last):
                nc.tensor.matmul(
                    pss[0],
                    lhsT=mt[:, off * 128:(off + 1) * 128],
                    rhs=ot[:, off * 128:(off + 1) * 128],
                    start=first, stop=last)

            def gcn_consume(t, pss):
                tmp = workp.tile([128, 128], dt.bfloat16, tag="tmp",
                                 name="tmp")
                nc.vector.tensor_tensor(
                    out=tmp, in0=pss[0],
                    in1=dinvrep_sb[:, t * 128:(t + 1) * 128], op=ALU.mult)
                aggsb = workp.tile([128, 128], dt.bfloat16, tag="aggsb",
                                   name="aggsb")
                nc.vector.tensor_tensor(
                    out=aggsb, in0=tmp,
                    in1=selfT_sb[:, t * 128:(t + 1) * 128], op=ALU.add)
                lp = psump.tile([128, 128], dt.float32, tag="layer",
                                name="lp_gcn")
                nc.tensor.matmul(lp, lhsT=aggsb, rhs=w0_sb,
                                 start=True, stop=False)
                nc.tensor.matmul(lp, lhsT=ones_sb[:, :128], rhs=b0_sb,
                                 start=False, stop=True)
                a1sb = workp.tile([128, 128], dt.bfloat16, tag="a1sb",
                                  name="a1sb")
                nc.scalar.activation(a1sb, lp, AF.Relu)
                nc.sync.dma_start(
                    out=cmb_dev[t * 128:(t + 1) * 128, D:2 * D], in_=a1sb)

            make_pass(gcn_layout, gcn_idx_sb, gcn_oh,
                      [xa_gcn[0:SPLIT, :], xa_gcn[SPLIT:N_A_PAD, :]],
                      128, "gcn", A_TILES, GCN_GROUP, 1,
                      gcn_chunk_mm, gcn_consume)

            # ========= phase 2: sliced AllGather of [xa | a1] =========
            for s in range(N_SLICES):
                r0 = int(SLICE_T0[s]) * 128
                r1 = int(SLICE_T0[s + 1]) * 128
                nc.gpsimd.collective_compute(
                    "AllGather", ALU.bypass,
                    replica_groups=[list(range(N_CORES))],
                    ins=[cmb_dev[r0:r1, :].opt()],
                    outs=[cmb_full[s].opt()],
                )

            # ================= phase 3: pooler -> xpwr =================
            blks = []
            b0_ = 0
            while b0_ < P_PC_PAD:
                bw = min(BLK, P_PC_PAD - b0_)
                blks.append((b0_, bw))
                b0_ += bw
            for h, (hb, bw) in enumerate(blks):
                clsT = clsp.tile([128, 6 * BLK], dt.bfloat16, tag="clsT",
                                 name=f"clsT_{h}")
                nc.gpsimd.dma_gather(
                    clsT[:, :6 * bw].rearrange("p (c e) -> p c e", e=bw),
                    emb, clsidx_sb[:, hb // 16:(hb + bw) // 16],
                    bw, bw, D_BERT, transpose=True, queue_num=next_q())
                xp_ps = pspoolp.tile([128, BLK], dt.float32, tag="xp",
                                     name=f"xp_ps{h}")
                for c in range(6):
                    po_ps = pspoolp.tile([128, BLK], dt.float32, tag="po",
                                         name=f"po_ps{h}_{c}")
                    for k in range(6):
                        nc.tensor.matmul(
                            po_ps[:, :bw],
                            lhsT=wp_sb[:, k * D_BERT + c * 128:
                                       k * D_BERT + (c + 1) * 128],
                            rhs=clsT[:, k * bw:(k + 1) * bw],
                            start=(k == 0), stop=False)
                    nc.tensor.matmul(
                        po_ps[:, :bw],
                        lhsT=bp_sb[:, c * 128:(c + 1) * 128],
                        rhs=ones_sb[:, :bw],
                        start=False, stop=True)
                    poT = clsp.tile([128, BLK], dt.bfloat16, tag="poT",
                                    name=f"poT{h}_{c}")
                    nc.scalar.activation(poT[:, :bw], po_ps[:, :bw], AF.Tanh)
                    nc.tensor.matmul(
                        xp_ps[:, :bw],
                        lhsT=wr0_sb[:, c * 128:(c + 1) * 128],
                        rhs=poT[:, :bw],
                        start=(c == 0), stop=False)
                    if c == 5:
                        nc.tensor.matmul(
                            xp_ps[:, :bw], lhsT=wr0f_sb,
                            rhs=featT_sb[:, hb:hb + bw],
                            start=False, stop=False)
                        nc.tensor.matmul(
                            xp_ps[:, :bw], lhsT=sb0_sb,
                            rhs=ones_sb[:, :bw],
                            start=False, stop=True)
                nc.vector.tensor_copy(xpwr[:, hb:hb + bw], xp_ps[:, :bw])

            # ================= phase 4: fused SAGE =================
            def sage_chunk_mm(pss, mt, ot, off, first, last):
                nc.tensor.matmul(
                    pss[0],
                    lhsT=mt[:, off * 256:off * 256 + 128],
                    rhs=ot[:, off * 128:(off + 1) * 128],
                    start=first, stop=last)
                nc.tensor.matmul(
                    pss[1],
                    lhsT=mt[:, off * 256 + 128:(off + 1) * 256],
                    rhs=ot[:, off * 128:(off + 1) * 128],
                    start=first, stop=last)

            def sage_consume(t, pss):
                mean0 = workp.tile([128, 128], dt.bfloat16, tag="mean0",
                                   name="mean0")
                nc.vector.tensor_tensor(
                    out=mean0, in0=pss[0],
                    in1=invcntrep_sb[:, t * 128:(t + 1) * 128], op=ALU.mult)
                mean1 = workp.tile([128, 128], dt.bfloat16, tag="mean1",
                                   name="mean1")
                nc.vector.tensor_tensor(
                    out=mean1, in0=pss[1],
                    in1=invcntrep_sb[:, t * 128:(t + 1) * 128], op=ALU.mult)
                pp = psump.tile([128, 128], dt.float32, tag="layer",
                                name="pp_s0")
                nc.tensor.matmul(pp, lhsT=wl0_sb, rhs=mean0,
                                 start=True, stop=False)
                nc.tensor.matmul(pp, lhsT=iden_sb,
                                 rhs=xpwr[:, t * 128:(t + 1) * 128],
                                 start=False, stop=True)
                nc.scalar.activation(p1T[:, t * 128:(t + 1) * 128], pp,
                                     AF.Relu)
                pq = psump.tile([128, 128], dt.float32, tag="layer",
                                name="pp_s1")
                nc.tensor.matmul(pq, lhsT=wl1_sb, rhs=mean1,
                                 start=True, stop=False)
                nc.tensor.matmul(pq, lhsT=wr1_sb,
                                 rhs=p1T[:, t * 128:(t + 1) * 128],
                                 start=False, stop=False)
                nc.tensor.matmul(pq, lhsT=sb1_sb, rhs=ones_sb[:, :128],
                                 start=False, stop=True)
                p2sb = workp.tile([128, 128], dt.bfloat16, tag="p2sb",
                                  name="p2sb")
                nc.scalar.activation(p2sb, pq, AF.Relu)
                hp_t = psump.tile([128, 128], dt.float32, tag="layer",
                                  name="hp")
                hp = hp_t[:, :OUT_D]
                nc.tensor.matmul(hp, lhsT=p2sb, rhs=linw_sb,
                                 start=True, stop=False)
                nc.tensor.matmul(hp, lhsT=ones_sb[:, :128], rhs=linb_sb,
                                 start=False, stop=True)
                outsb = workp.tile([128, OUT_D], dt.float32, tag="outsb",
                                   name="outsb")
                nc.vector.tensor_copy(outsb, hp)
                nc.sync.dma_start(out=out[t * 128:(t + 1) * 128, :],
                                  in_=outsb)

            make_pass(sage_layout, sage_idx_sb, sage_oh,
                      [cf[:, :] for cf in cmb_full],
                      256, "sage", P_TILES, SAGE_GROUP, 2,
                      sage_chunk_mm, sage_consume)

    nc.compile()
    return nc


def kernel(**inputs):
    from concourse import bass_utils

    in_maps, (gcn_layout, sage_layout) = _prep(inputs)

    key = (gcn_layout["total_chunks"], sage_layout["total_chunks"])
    if key in _CACHE:
        nc = _CACHE[key]
    else:
        nc = _build(gcn_layout, sage_layout)
        _CACHE[key] = nc

    kw = {}
    if TRACE:
        import os
        import shutil
        shutil.rmtree("/tmp/neff_trace", ignore_errors=True)
        os.makedirs("/tmp/neff_trace", exist_ok=True)
        kw = dict(tmpdir="/tmp/neff_trace")
    res = bass_utils.run_bass_kernel_spmd(
        nc, in_maps, core_ids=list(range(N_CORES)), trace=TRACE, **kw)
    LAST["exec_time_ns"] = res.exec_time_ns
    LAST["results"] = res

    pieces = [res.results[c]["out"][:P_PC] for c in range(N_CORES)]
    return np.concatenate(pieces, axis=0).astype(np.float32)


# revision 12
# speedup vs baseline: 3.2647x; 2.4842x over previous
"""HeteroGNN (GCN over authors + SAGE over papers) on 8 Trainium2 NeuronCores.

v2 design (from trace analysis of the v1 baseline, which was ~3.1 ms and
99% GPSIMD-bound on dma_gather descriptor generation):
  - dma_gather calls round-robin over 4 SWDGE queues (queue_num selects the
    Q7 core pair that generates descriptors -> ~3x parallel desc-gen).
  - sage0+sage1 share ONE gather per writes-edge: 512B rows from a combined
    [xa | a1] table, AllGathered in 4 tile-sliced collectives so the
    collective overlaps the GCN tail; sliced tables also fit int16 indices
    without a lo/hi split.
  - Segment-sum one-hots are PURE 0/1: GCN's per-edge norm is folded into a
    pre-scaled gather table (dinv[src]*xa) plus a per-tile dst post-scale;
    SAGE's 1/cnt is a per-tile dst post-scale.  Pure one-hots are
    host-staged in fp8 and DMA'd via HWDGE (nc.sync) - PE accepts
    bf16 lhsT x fp8 rhs exactly (verified on HW).
  - GCN self-loops: host-staged self term added during the psum->SBUF copy.
  - Pooler GEMM restructured weight-stationary with 512-wide streams; its
    output is pre-multiplied by sage_Wr0 into xpwr so the SAGE loop only
    adds a slice.
"""

import sys

sys.path.insert(0, "/opt/trn_rl_repo")

import numpy as np
import ml_dtypes

BF16 = ml_dtypes.bfloat16
FP8 = ml_dtypes.float8_e4m3

# ---- problem constants ----
N_CORES = 8
N_AUTHOR = 50000
N_PAPER = 25000
D = 128
D_BERT = 768
FEAT = 12
OUT_D = 64
VOCAB = 31090

A_PC = N_AUTHOR // N_CORES      # 6250
A_TILES = 49
A_PC_PAD = A_TILES * 128        # 6272
N_A_PAD = N_CORES * A_PC_PAD    # 50176
P_PC = N_PAPER // N_CORES       # 3125
P_TILES = 25
P_PC_PAD = P_TILES * 128        # 3200
SPLIT = 32768                   # int16 idx limit for the GCN table
CALL_CHUNKS = 8                 # max 1024 rows per dma_gather call
N_QUEUES = 4

N_SLICES = 4                    # AllGather slices (author tile ranges)
SLICE_TILES = [13, 13, 13, 10]
SLICE_T0 = np.concatenate([[0], np.cumsum(SLICE_TILES)]).astype(np.int64)

GCN_GROUP = 4                   # dst tiles whose agg psums are open together
SAGE_GROUP = 2

BLK = 512                       # pooler block (psum + cls-gather granularity)

TRACE = False
LAST = {}
_CACHE = {}


def _wrap_idx(flat):
    n = flat.shape[0]
    assert n % 16 == 0
    w = flat.reshape(n // 16, 16).T.astype(np.int16)
    return np.tile(w, (8, 1))


def _pad_author_ids(v):
    return (v // A_PC) * A_PC_PAD + (v % A_PC)


def _common_layout(counts_list, n_tiles, n_tabs, group):
    """SPMD-common chunk layout: per (tile, table) the max chunk count over
    cores; runs ordered [group: [table: [tile: run]]]; calls merge adjacent
    same-table chunks up to CALL_CHUNKS."""
    nch = np.zeros((n_tiles, n_tabs), dtype=np.int64)
    for counts in counts_list:
        nch = np.maximum(nch, -(-counts // 128))

    runs = []                      # (tile, table, chunk0, n)
    tile_runs = [[] for _ in range(n_tiles)]
    cursor = 0
    for g0 in range(0, n_tiles, group):
        g1 = min(g0 + group, n_tiles)
        for tb in range(n_tabs):
            for t in range(g0, g1):
                n = int(nch[t, tb])
                if n == 0:
                    continue
                runs.append((t, tb, cursor, n))
                tile_runs[t].append((cursor, n))
                cursor += n

    calls = []
    for t, tb, c0, n in runs:
        if calls and calls[-1][0] == tb and \
                calls[-1][1] + calls[-1][2] == c0 and \
                calls[-1][2] + n <= CALL_CHUNKS:
            calls[-1] = (tb, calls[-1][1], calls[-1][2] + n)
        else:
            while n > CALL_CHUNKS:
                calls.append((tb, c0, CALL_CHUNKS))
                c0 += CALL_CHUNKS
                n -= CALL_CHUNKS
            if n:
                calls.append((tb, c0, n))
    # split any call that grew past the cap via merging
    final = []
    for tb, c0, n in calls:
        while n > CALL_CHUNKS:
            final.append((tb, c0, CALL_CHUNKS))
            c0 += CALL_CHUNKS
            n -= CALL_CHUNKS
        if n:
            final.append((tb, c0, n))
    return dict(nch=nch, runs=runs, tile_runs=tile_runs, calls=final,
                total_chunks=cursor)


def _fill_core(layout, srow, tab, tile, drel):
    """Lay this core's edges into the common layout.  Returns
    (idx_flat [TOT*128] int64, oh [TOT*128, 128] fp8)."""
    tot = layout["total_chunks"]
    idx_flat = np.zeros(tot * 128, dtype=np.int64)
    dr_all = np.full(tot * 128, -1, dtype=np.int64)
    for t, tb, c0, n in layout["runs"]:
        m = (tile == t) & (tab == tb)
        e = np.nonzero(m)[0]
        ne = len(e)
        assert ne <= n * 128
        idx_flat[c0 * 128: c0 * 128 + ne] = srow[e]
        dr_all[c0 * 128: c0 * 128 + ne] = drel[e]
    oh = np.zeros((tot * 128, 128), dtype=FP8)
    rr = np.nonzero(dr_all >= 0)[0]
    oh[rr, dr_all[rr]] = np.float32(1.0)
    return idx_flat, oh


def _prep(inputs):
    f32 = np.float32
    x_author = np.asarray(inputs["x_author"], f32)
    paper_tokens = np.asarray(inputs["paper_tokens"])
    paper_feat = np.asarray(inputs["paper_feat"], f32)
    edge_collab = np.asarray(inputs["edge_collab"], np.int64)
    writes_src = np.asarray(inputs["writes_src"], np.int64)
    writes_dst = np.asarray(inputs["writes_dst"], np.int64)

    src_c, dst_c = edge_collab[0], edge_collab[1]
    deg = np.bincount(dst_c, minlength=N_AUTHOR).astype(f32) + 1.0
    dinv = 1.0 / np.sqrt(deg)

    rows = _pad_author_ids(np.arange(N_AUTHOR))
    xa_gcn = np.zeros((N_A_PAD, D), dtype=BF16)       # dinv[v] * xa[v]
    xa_gcn[rows] = (x_author * dinv[:, None]).astype(BF16)
    cnt = np.bincount(writes_dst, minlength=N_PAPER).astype(f32)
    invcnt = 1.0 / np.maximum(cnt, 1.0)

    emb = np.asarray(inputs["embed_table"], f32).astype(BF16)

    # ---- GCN edge partition (per core, common layout)
    src_c_pad = _pad_author_ids(src_c)
    gcn_cores = []
    gcn_counts = []
    for c in range(N_CORES):
        m = (dst_c >= c * A_PC) & (dst_c < (c + 1) * A_PC)
        s = src_c_pad[m]
        dloc = dst_c[m] - c * A_PC
        tab = (s >= SPLIT).astype(np.int64)
        srow = np.where(tab == 0, s, s - SPLIT)
        tile = dloc >> 7
        drel = dloc & 127
        gcn_cores.append((srow, tab, tile, drel))
        cnts = np.zeros((A_TILES, 2), dtype=np.int64)
        np.add.at(cnts, (tile, tab), 1)
        gcn_counts.append(cnts)
    gcn_layout = _common_layout(gcn_counts, A_TILES, 2, GCN_GROUP)

    # ---- SAGE edge partition: table id = AllGather stripe
    ws_pad = _pad_author_ids(writes_src)
    ws_block = ws_pad // A_PC_PAD
    ws_local = ws_pad % A_PC_PAD
    ws_tile = ws_local >> 7
    ws_stripe = np.searchsorted(SLICE_T0[1:], ws_tile, side="right")
    stripe_rows = np.array([n * 128 for n in SLICE_TILES])
    off_rows = SLICE_T0[:-1] * 128
    ws_row = ws_block * stripe_rows[ws_stripe] + (ws_local - off_rows[ws_stripe])
    sage_cores = []
    sage_counts = []
    for c in range(N_CORES):
        m = (writes_dst >= c * P_PC) & (writes_dst < (c + 1) * P_PC)
        dloc = writes_dst[m] - c * P_PC
        tile = dloc >> 7
        drel = dloc & 127
        sage_cores.append((ws_row[m], ws_stripe[m], tile, drel))
        cnts = np.zeros((P_TILES, N_SLICES), dtype=np.int64)
        np.add.at(cnts, (tile, ws_stripe[m]), 1)
        sage_counts.append(cnts)
    sage_layout = _common_layout(sage_counts, P_TILES, N_SLICES, SAGE_GROUP)

    # ---- weights
    def brow(name, n):
        return np.asarray(inputs[name], f32).reshape(1, n).astype(BF16)

    Wp = np.asarray(inputs["Wp"], f32)
    wp_sb = Wp.reshape(6, 128, D_BERT).transpose(1, 0, 2).reshape(128, 6 * D_BERT)
    Wr0 = np.asarray(inputs["sage_Wr0"], f32)
    wr0_sb = Wr0[:768].reshape(6, 128, 128).transpose(1, 0, 2).reshape(128, 6 * 128)
    wr0f = Wr0[768:780]

    consts = dict(
        wp=wp_sb.astype(BF16),
        bp=brow("bp", D_BERT),
        w0=np.asarray(inputs["gcn_W0"], f32).astype(BF16),
        b0=brow("gcn_b0", D),
        wl0=np.asarray(inputs["sage_Wl0"], f32).astype(BF16),
        wr0=wr0_sb.astype(BF16),
        wr0f=wr0f.astype(BF16),
        sb0=brow("sage_b0", D),
        wl1=np.asarray(inputs["sage_Wl1"], f32).astype(BF16),
        wr1=np.asarray(inputs["sage_Wr1"], f32).astype(BF16),
        sb1=brow("sage_b1", D),
        linw=np.asarray(inputs["lin_W"], f32).astype(BF16),
        linb=brow("lin_b", OUT_D),
        ones=np.ones((1, 512), dtype=BF16),
        iden=np.eye(128, dtype=BF16),
    )

    in_maps = []
    for c in range(N_CORES):
        a0 = c * A_PC
        dv = np.zeros(A_PC_PAD, f32)
        dv[:A_PC] = dinv[a0:a0 + A_PC]
        dinvrep = np.tile(dv[None, :].astype(BF16), (128, 1))
        selfT = np.zeros((D, A_PC_PAD), dtype=BF16)
        selfT[:, :A_PC] = (x_author[a0:a0 + A_PC] *
                           (dv[:A_PC] ** 2)[:, None]).T.astype(BF16)
        cmb_shard = np.zeros((A_PC_PAD, 2 * D), dtype=BF16)
        cmb_shard[:A_PC, :D] = x_author[a0:a0 + A_PC].astype(BF16)

        p0 = c * P_PC
        iv = np.zeros(P_PC_PAD, f32)
        iv[:P_PC] = invcnt[p0:p0 + P_PC]
        invcntrep = np.tile(iv[None, :].astype(BF16), (128, 1))

        cls = np.zeros(P_PC_PAD, dtype=np.int64)
        cls[:P_PC] = paper_tokens[p0:p0 + P_PC, 0]
        featT = np.zeros((FEAT, P_PC_PAD), dtype=BF16)
        featT[:, :P_PC] = paper_feat[p0:p0 + P_PC].T.astype(BF16)

        gidx, goh = _fill_core(gcn_layout, *gcn_cores[c])
        sidx, soh = _fill_core(sage_layout, *sage_cores[c])

        m = dict(
            emb=emb,
            xa_gcn=xa_gcn,
            cmb_shard=cmb_shard,
            clsidx=_wrap_idx(cls),
            featT=featT,
            dinvrep=dinvrep,
            selfT=selfT,
            invcntrep=invcntrep,
            gcn_idx=_wrap_idx(gidx),
            gcn_oh=goh,
            sage_idx=_wrap_idx(sidx),
            sage_oh=soh,
            **consts,
        )
        in_maps.append(m)

    return in_maps, (gcn_layout, sage_layout)


def _build(gcn_layout, sage_layout):
    import concourse.bacc as bacc
    import concourse.mybir as mybir
    from concourse.tile import TileContext

    dt = mybir.dt
    AF = mybir.ActivationFunctionType
    ALU = mybir.AluOpType

    GCN_CH = gcn_layout["total_chunks"]
    SAGE_CH = sage_layout["total_chunks"]

    nc = bacc.Bacc("TRN2", target_bir_lowering=False, debug=False,
                   num_devices=N_CORES, num_swdge_queues=N_QUEUES)

    def din(name, shape, dtype=dt.bfloat16):
        return nc.dram_tensor(name, list(shape), dtype, kind="ExternalInput").ap()

    emb = din("emb", (VOCAB, D_BERT))
    xa_gcn = din("xa_gcn", (N_A_PAD, D))
    cmb_shard = din("cmb_shard", (A_PC_PAD, 2 * D))
    clsidx = din("clsidx", (128, P_PC_PAD // 16), dt.int16)
    featT = din("featT", (FEAT, P_PC_PAD))
    dinvrep = din("dinvrep", (128, A_PC_PAD))
    selfT = din("selfT", (D, A_PC_PAD))
    invcntrep = din("invcntrep", (128, P_PC_PAD))
    gcn_idx = din("gcn_idx", (128, GCN_CH * 8), dt.int16)
    gcn_oh = din("gcn_oh", (GCN_CH * 128, 128), dt.float8e4)
    sage_idx = din("sage_idx", (128, SAGE_CH * 8), dt.int16)
    sage_oh = din("sage_oh", (SAGE_CH * 128, 128), dt.float8e4)
    wp = din("wp", (128, 6 * D_BERT))
    bp = din("bp", (1, D_BERT))
    w0 = din("w0", (D, D))
    b0 = din("b0", (1, D))
    wl0 = din("wl0", (D, D))
    wr0 = din("wr0", (128, 6 * 128))
    wr0f = din("wr0f", (FEAT, D))
    sb0 = din("sb0", (1, D))
    wl1 = din("wl1", (D, D))
    wr1 = din("wr1", (D, D))
    sb1 = din("sb1", (1, D))
    linw = din("linw", (D, OUT_D))
    linb = din("linb", (1, OUT_D))
    ones = din("ones", (1, 512))
    iden = din("iden", (128, 128))
    out = nc.dram_tensor("out", [P_PC_PAD, OUT_D], dt.float32,
                         kind="ExternalOutput").ap()

    with TileContext(nc) as tc:
        with (
            tc.tile_pool(name="const", bufs=1) as constp,
            tc.tile_pool(name="sticky", bufs=1) as stickyp,
            tc.tile_pool(name="cls", bufs=2) as clsp,
            tc.tile_pool(name="msg", bufs=10) as msgp,
            tc.tile_pool(name="ohp", bufs=10) as ohp,
            tc.tile_pool(name="work", bufs=4) as workp,
            tc.tile_pool(name="psum", bufs=2, space="PSUM") as psump,
            tc.tile_pool(name="psagg", bufs=4, space="PSUM") as psaggp,
            tc.tile_pool(name="pspool", bufs=1, space="PSUM") as pspoolp,
            tc.tile_pool(name="dram", bufs=1, space="DRAM") as dramp,
        ):
            def load_const(ap_dram, name):
                t = constp.tile(list(ap_dram.shape), ap_dram.dtype, name=name)
                nc.sync.dma_start(out=t, in_=ap_dram)
                return t

            w0_sb = load_const(w0, "w0_sb")
            b0_sb = load_const(b0, "b0_sb")
            wl0_sb = load_const(wl0, "wl0_sb")
            wr0_sb = load_const(wr0, "wr0_sb")
            wr0f_sb = load_const(wr0f, "wr0f_sb")
            sb0_sb = load_const(sb0, "sb0_sb")
            wl1_sb = load_const(wl1, "wl1_sb")
            wr1_sb = load_const(wr1, "wr1_sb")
            sb1_sb = load_const(sb1, "sb1_sb")
            linw_sb = load_const(linw, "linw_sb")
            linb_sb = load_const(linb, "linb_sb")
            ones_sb = load_const(ones, "ones_sb")
            iden_sb = load_const(iden, "iden_sb")
            wp_sb = load_const(wp, "wp_sb")
            bp_sb = load_const(bp, "bp_sb")
            featT_sb = load_const(featT, "featT_sb")
            clsidx_sb = load_const(clsidx, "clsidx_sb")
            dinvrep_sb = load_const(dinvrep, "dinvrep_sb")
            selfT_sb = load_const(selfT, "selfT_sb")
            invcntrep_sb = load_const(invcntrep, "invcntrep_sb")
            gcn_idx_sb = load_const(gcn_idx, "gcn_idx_sb")
            sage_idx_sb = load_const(sage_idx, "sage_idx_sb")

            xpwr = stickyp.tile([128, P_PC_PAD], dt.bfloat16, name="xpwr")
            p1T = stickyp.tile([128, P_PC_PAD], dt.bfloat16, name="p1T")

            cmb_full = [
                dramp.tile([N_CORES * SLICE_TILES[s] * 128, 2 * D],
                           dt.bfloat16, addr_space="Shared",
                           name=f"cmb_full{s}")
                for s in range(N_SLICES)
            ]
            # device-owned shard buffer (Tile tracks deps on pool tiles);
            # xa half copied from the host-staged input, a1 half written
            # by the GCN consume step.
            cmb_dev = dramp.tile([A_PC_PAD, 2 * D], dt.bfloat16,
                                 name="cmb_dev")
            nc.sync.dma_start(out=cmb_dev[:, 0:D], in_=cmb_shard[:, 0:D])

            # ================= pass machinery =================
            def make_pass(layout, idx_sb, oh_dram, tables, elem, pname,
                          n_tiles, group, n_psums, chunk_mm, consume):
                calls = layout["calls"]
                tile_runs = layout["tile_runs"]
                chunk_loc = {}
                for k, (tb, c0, nchk) in enumerate(calls):
                    for i in range(nchk):
                        chunk_loc[c0 + i] = (k, i)
                msg_tiles = {}
                oh_tiles = {}

                def ensure_call(k):
                    tb, c0, nchk = calls[k]
                    mt = msgp.tile([128, CALL_CHUNKS * elem], dt.bfloat16,
                                   tag="msg", name=f"msg_{pname}_{k}")
                    nc.gpsimd.dma_gather(
                        mt[:, :nchk * elem].rearrange(
                            "p (c e) -> p c e", e=elem),
                        tables[tb], idx_sb[:, c0 * 8:(c0 + nchk) * 8],
                        nchk * 128, nchk * 128, elem)
                    ot = ohp.tile([128, CALL_CHUNKS * 128], dt.float8e4,
                                  tag="oh", name=f"oh_{pname}_{k}")
                    nc.sync.dma_start(
                        out=ot[:, :nchk * 128].rearrange(
                            "p (c e) -> p c e", e=128),
                        in_=oh_dram[c0 * 128:(c0 + nchk) * 128, :].rearrange(
                            "(c p) e -> p c e", p=128))
                    msg_tiles[k] = mt
                    oh_tiles[k] = ot

                issued = [0]

                def issue_upto(k):
                    while issued[0] <= min(k + 2, len(calls) - 1):
                        ensure_call(issued[0])
                        issued[0] += 1

                g0 = 0
                while g0 < n_tiles:
                    g1 = min(g0 + group, n_tiles)
                    last_chunk = max(
                        r0 + nr - 1
                        for t in range(g0, g1) for (r0, nr) in tile_runs[t])
                    issue_upto(chunk_loc[last_chunk][0])
                    for t in range(g0, g1):
                        pss = [psaggp.tile([128, 128], dt.float32, tag="agg",
                                           name=f"agg_{pname}_{t}_{j}")
                               for j in range(n_psums)]
                        runs = tile_runs[t]
                        n_run_chunks = sum(nr for _, nr in runs)
                        ci = 0
                        for (r0, nr) in runs:
                            for i in range(nr):
                                k, off = chunk_loc[r0 + i]
                                chunk_mm(pss, msg_tiles[k], oh_tiles[k], off,
                                         first=(ci == 0),
                                         last=(ci == n_run_chunks - 1))
                                ci += 1
                        consume(t, pss)
                    g0 = g1

            # ================= phase 1: GCN =================
            def gcn_chunk_mm(pss, mt, ot, off, first, last):
                nc.tensor.matmul(
                    pss[0],
                    lhsT=mt[:, off * 128:(off + 1) * 128],
                    rhs=ot[:, off * 128:(off + 1) * 128],
                    start=first, stop=last)

            def gcn_consume(t, pss):
                tmp = workp.tile([128, 128], dt.bfloat16, tag="tmp",
                                 name="tmp")
                nc.vector.tensor_tensor(
                    out=tmp, in0=pss[0],
                    in1=dinvrep_sb[:, t * 128:(t + 1) * 128], op=ALU.mult)
                aggsb = workp.tile([128, 128], dt.bfloat16, tag="aggsb",
                                   name="aggsb")
                nc.vector.tensor_tensor(
                    out=aggsb, in0=tmp,
                    in1=selfT_sb[:, t * 128:(t + 1) * 128], op=ALU.add)
                lp = psump.tile([128, 128], dt.float32, tag="layer",
                                name="lp_gcn")
                nc.tensor.matmul(lp, lhsT=aggsb, rhs=w0_sb,
                                 start=True, stop=False)
                nc.tensor.matmul(lp, lhsT=ones_sb[:, :128], rhs=b0_sb,
                                 start=False, stop=True)
                a1sb = workp.tile([128, 128], dt.bfloat16, tag="a1sb",
                                  name="a1sb")
                nc.scalar.activation(a1sb, lp, AF.Relu)
                nc.sync.dma_start(
                    out=cmb_dev[t * 128:(t + 1) * 128, D:2 * D], in_=a1sb)

            make_pass(gcn_layout, gcn_idx_sb, gcn_oh,
                      [xa_gcn[0:SPLIT, :], xa_gcn[SPLIT:N_A_PAD, :]],
                      128, "gcn", A_TILES, GCN_GROUP, 1,
                      gcn_chunk_mm, gcn_consume)

            # ========= phase 2: sliced AllGather of [xa | a1] =========
            for s in range(N_SLICES):
                r0 = int(SLICE_T0[s]) * 128
                r1 = int(SLICE_T0[s + 1]) * 128
                nc.gpsimd.collective_compute(
                    "AllGather", ALU.bypass,
                    replica_groups=[list(range(N_CORES))],
                    ins=[cmb_dev[r0:r1, :].opt()],
                    outs=[cmb_full[s].opt()],
                )

            # ================= phase 3: pooler -> xpwr =================
            blks = []
            b0_ = 0
            while b0_ < P_PC_PAD:
                bw = min(BLK, P_PC_PAD - b0_)
                blks.append((b0_, bw))
                b0_ += bw
            for h, (hb, bw) in enumerate(blks):
                clsT = clsp.tile([128, 6 * BLK], dt.bfloat16, tag="clsT",
                                 name=f"clsT_{h}")
                nc.gpsimd.dma_gather(
                    clsT[:, :6 * bw].rearrange("p (c e) -> p c e", e=bw),
                    emb, clsidx_sb[:, hb // 16:(hb + bw) // 16],
                    bw, bw, D_BERT, transpose=True)
                xp_ps = pspoolp.tile([128, BLK], dt.float32, tag="xp",
                                     name=f"xp_ps{h}")
                for c in range(6):
                    po_ps = pspoolp.tile([128, BLK], dt.float32, tag="po",
                                         name=f"po_ps{h}_{c}")
                    for k in range(6):
                        nc.tensor.matmul(
                            po_ps[:, :bw],
                            lhsT=wp_sb[:, k * D_BERT + c * 128:
                                       k * D_BERT + (c + 1) * 128],
                            rhs=clsT[:, k * bw:(k + 1) * bw],
                            start=(k == 0), stop=False)
                    nc.tensor.matmul(
                        po_ps[:, :bw],
                        lhsT=bp_sb[:, c * 128:(c + 1) * 128],
                        rhs=ones_sb[:, :bw],
                        start=False, stop=True)
                    poT = clsp.tile([128, BLK], dt.bfloat16, tag="poT",
                                    name=f"poT{h}_{c}")
                    nc.scalar.activation(poT[:, :bw], po_ps[:, :bw], AF.Tanh)
                    nc.tensor.matmul(
                        xp_ps[:, :bw],
                        lhsT=wr0_sb[:, c * 128:(c + 1) * 128],
                        rhs=poT[:, :bw],
                        start=(c == 0), stop=False)
                    if c == 5:
                        nc.tensor.matmul(
                            xp_ps[:, :bw], lhsT=wr0f_sb,
                            rhs=featT_sb[:, hb:hb + bw],
                            start=False, stop=False)
                        nc.tensor.matmul(
                            xp_ps[:, :bw], lhsT=sb0_sb,
                            rhs=ones_sb[:, :bw],
                            start=False, stop=True)
                nc.vector.tensor_copy(xpwr[:, hb:hb + bw], xp_ps[:, :bw])

            # ================= phase 4: fused SAGE =================
            def sage_chunk_mm(pss, mt, ot, off, first, last):
                nc.tensor.matmul(
                    pss[0],
                    lhsT=mt[:, off * 256:off * 256 + 128],
                    rhs=ot[:, off * 128:(off + 1) * 128],
                    start=first, stop=last)
                nc.tensor.matmul(
                    pss[1],
                    lhsT=mt[:, off * 256 + 128:(off + 1) * 256],
                    rhs=ot[:, off * 128:(off + 1) * 128],
                    start=first, stop=last)

            def sage_consume(t, pss):
                mean0 = workp.tile([128, 128], dt.bfloat16, tag="mean0",
                                   name="mean0")
                nc.vector.tensor_tensor(
                    out=mean0, in0=pss[0],
                    in1=invcntrep_sb[:, t * 128:(t + 1) * 128], op=ALU.mult)
                mean1 = workp.tile([128, 128], dt.bfloat16, tag="mean1",
                                   name="mean1")
                nc.vector.tensor_tensor(
                    out=mean1, in0=pss[1],
                    in1=invcntrep_sb[:, t * 128:(t + 1) * 128], op=ALU.mult)
                pp = psump.tile([128, 128], dt.float32, tag="layer",
                                name="pp_s0")
                nc.tensor.matmul(pp, lhsT=wl0_sb, rhs=mean0,
                                 start=True, stop=False)
                nc.tensor.matmul(pp, lhsT=iden_sb,
                                 rhs=xpwr[:, t * 128:(t + 1) * 128],
                                 start=False, stop=True)
                nc.scalar.activation(p1T[:, t * 128:(t + 1) * 128], pp,
                                     AF.Relu)
                pq = psump.tile([128, 128], dt.float32, tag="layer",
                                name="pp_s1")
                nc.tensor.matmul(pq, lhsT=wl1_sb, rhs=mean1,
                                 start=True, stop=False)
                nc.tensor.matmul(pq, lhsT=wr1_sb,
                                 rhs=p1T[:, t * 128:(t + 1) * 128],
                                 start=False, stop=False)
                nc.tensor.matmul(pq, lhsT=sb1_sb, rhs=ones_sb[:, :128],
                                 start=False, stop=True)
                p2sb = workp.tile([128, 128], dt.bfloat16, tag="p2sb",
                                  name="p2sb")
                nc.scalar.activation(p2sb, pq, AF.Relu)
                hp_t = psump.tile([128, 128], dt.float32, tag="layer",
                                  name="hp")
                hp = hp_t[:, :OUT_D]
                nc.tensor.matmul(hp, lhsT=p2sb, rhs=linw_sb,
                                 start=True, stop=False)
                nc.tensor.matmul(hp, lhsT=ones_sb[:, :128], rhs=linb_sb,
                                 start=False, stop=True)
                outsb = workp.tile([128, OUT_D], dt.float32, tag="outsb",
                                   name="outsb")
                nc.vector.tensor_copy(outsb, hp)
                nc.sync.dma_start(out=out[t * 128:(t + 1) * 128, :],
                                  in_=outsb)

            make_pass(sage_layout, sage_idx_sb, sage_oh,
                      [cf[:, :] for cf in cmb_full],
                      256, "sage", P_TILES, SAGE_GROUP, 2,
                      sage_chunk_mm, sage_consume)

    # Align each SWDGE gather's queue with its Tile DMASW semaphore lane:
    # lane completions must be FIFO for Tile's cumulative sem targets, and
    # per-queue rings complete FIFO, so queue = lane % N_QUEUES keeps every
    # lane single-queue while spreading desc-gen over 4 Q7 core pairs.
    for b in nc.m.functions[0].blocks:
        for inst in b.instructions:
            if isinstance(inst, mybir.InstDMAGatherAnt):
                proc = getattr(inst, "bass_scheduled_proc", None)
                if proc is not None and 11 <= proc <= 18:
                    inst.queue_num = (proc - 11) % N_QUEUES

    nc.compile()
    return nc


def kernel(**inputs):
    from concourse import bass_utils

    in_maps, (gcn_layout, sage_layout) = _prep(inputs)

    key = (gcn_layout["total_chunks"], sage_layout["total_chunks"])
    if key in _CACHE:
        nc = _CACHE[key]
    else:
        nc = _build(gcn_layout, sage_layout)
        _CACHE[key] = nc

    kw = {}
    if TRACE:
        import os
        import shutil
        shutil.rmtree("/tmp/neff_trace", ignore_errors=True)
        os.makedirs("/tmp/neff_trace", exist_ok=True)
        kw = dict(tmpdir="/tmp/neff_trace")
    res = bass_utils.run_bass_kernel_spmd(
        nc, in_maps, core_ids=list(range(N_CORES)), trace=TRACE, **kw)
    LAST["exec_time_ns"] = res.exec_time_ns
    LAST["results"] = res

    pieces = [res.results[c]["out"][:P_PC] for c in range(N_CORES)]
    return np.concatenate(pieces, axis=0).astype(np.float32)


# revision 13
# speedup vs baseline: 3.3804x; 1.0354x over previous
"""HeteroGNN (GCN over authors + SAGE over papers) on 8 Trainium2 NeuronCores.

v2 design (from trace analysis of the v1 baseline, which was ~3.1 ms and
99% GPSIMD-bound on dma_gather descriptor generation):
  - dma_gather calls round-robin over 4 SWDGE queues (queue_num selects the
    Q7 core pair that generates descriptors -> ~3x parallel desc-gen).
  - sage0+sage1 share ONE gather per writes-edge: 512B rows from a combined
    [xa | a1] table, AllGathered in 4 tile-sliced collectives so the
    collective overlaps the GCN tail; sliced tables also fit int16 indices
    without a lo/hi split.
  - Segment-sum one-hots are PURE 0/1: GCN's per-edge norm is folded into a
    pre-scaled gather table (dinv[src]*xa) plus a per-tile dst post-scale;
    SAGE's 1/cnt is a per-tile dst post-scale.  Pure one-hots are
    host-staged in fp8 and DMA'd via HWDGE (nc.sync) - PE accepts
    bf16 lhsT x fp8 rhs exactly (verified on HW).
  - GCN self-loops: host-staged self term added during the psum->SBUF copy.
  - Pooler GEMM restructured weight-stationary with 512-wide streams; its
    output is pre-multiplied by sage_Wr0 into xpwr so the SAGE loop only
    adds a slice.
"""

import sys

sys.path.insert(0, "/opt/trn_rl_repo")

import numpy as np
import ml_dtypes

BF16 = ml_dtypes.bfloat16
FP8 = ml_dtypes.float8_e4m3

# ---- problem constants ----
N_CORES = 8
N_AUTHOR = 50000
N_PAPER = 25000
D = 128
D_BERT = 768
FEAT = 12
OUT_D = 64
VOCAB = 31090

A_PC = N_AUTHOR // N_CORES      # 6250
A_TILES = 49
A_PC_PAD = A_TILES * 128        # 6272
N_A_PAD = N_CORES * A_PC_PAD    # 50176
P_PC = N_PAPER // N_CORES       # 3125
P_TILES = 25
P_PC_PAD = P_TILES * 128        # 3200
SPLIT = 32768                   # int16 idx limit for the GCN table
CALL_CHUNKS = 8                 # max 1024 rows per dma_gather call
N_QUEUES = 4

N_SLICES = 4                    # AllGather slices (author tile ranges)
SLICE_TILES = [13, 13, 13, 10]
SLICE_T0 = np.concatenate([[0], np.cumsum(SLICE_TILES)]).astype(np.int64)

GCN_GROUP = 4                   # dst tiles whose agg psums are open together
SAGE_GROUP = 2

BLK = 512                       # pooler block (psum + cls-gather granularity)

TRACE = False
LAST = {}
_CACHE = {}


def _wrap_idx(flat):
    n = flat.shape[0]
    assert n % 16 == 0
    w = flat.reshape(n // 16, 16).T.astype(np.int16)
    return np.tile(w, (8, 1))


def _pad_author_ids(v):
    return (v // A_PC) * A_PC_PAD + (v % A_PC)


def _common_layout(counts_list, n_tiles, n_tabs, group):
    """SPMD-common chunk layout: per (tile, table) the max chunk count over
    cores; runs ordered [group: [table: [tile: run]]]; calls merge adjacent
    same-table chunks up to CALL_CHUNKS."""
    nch = np.zeros((n_tiles, n_tabs), dtype=np.int64)
    for counts in counts_list:
        nch = np.maximum(nch, -(-counts // 128))

    runs = []                      # (tile, table, chunk0, n)
    tile_runs = [[] for _ in range(n_tiles)]
    cursor = 0
    for g0 in range(0, n_tiles, group):
        g1 = min(g0 + group, n_tiles)
        for tb in range(n_tabs):
            for t in range(g0, g1):
                n = int(nch[t, tb])
                if n == 0:
                    continue
                runs.append((t, tb, cursor, n))
                tile_runs[t].append((cursor, n))
                cursor += n

    calls = []
    for t, tb, c0, n in runs:
        if calls and calls[-1][0] == tb and \
                calls[-1][1] + calls[-1][2] == c0 and \
                calls[-1][2] + n <= CALL_CHUNKS:
            calls[-1] = (tb, calls[-1][1], calls[-1][2] + n)
        else:
            while n > CALL_CHUNKS:
                calls.append((tb, c0, CALL_CHUNKS))
                c0 += CALL_CHUNKS
                n -= CALL_CHUNKS
            if n:
                calls.append((tb, c0, n))
    # split any call that grew past the cap via merging
    final = []
    for tb, c0, n in calls:
        while n > CALL_CHUNKS:
            final.append((tb, c0, CALL_CHUNKS))
            c0 += CALL_CHUNKS
            n -= CALL_CHUNKS
        if n:
            final.append((tb, c0, n))
    return dict(nch=nch, runs=runs, tile_runs=tile_runs, calls=final,
                total_chunks=cursor)


def _fill_core(layout, srow, tab, tile, drel):
    """Lay this core's edges into the common layout.  Returns
    (idx_flat [TOT*128] int64, oh [TOT*128, 128] fp8)."""
    tot = layout["total_chunks"]
    idx_flat = np.zeros(tot * 128, dtype=np.int64)
    dr_all = np.full(tot * 128, -1, dtype=np.int64)
    for t, tb, c0, n in layout["runs"]:
        m = (tile == t) & (tab == tb)
        e = np.nonzero(m)[0]
        ne = len(e)
        assert ne <= n * 128
        idx_flat[c0 * 128: c0 * 128 + ne] = srow[e]
        dr_all[c0 * 128: c0 * 128 + ne] = drel[e]
    oh = np.zeros((tot * 128, 128), dtype=FP8)
    rr = np.nonzero(dr_all >= 0)[0]
    oh[rr, dr_all[rr]] = np.float32(1.0)
    # partition-major for contiguous per-partition DMA: [128p, tot, 128d]
    oh = np.ascontiguousarray(
        oh.reshape(tot, 128, 128).transpose(1, 0, 2)).reshape(128, tot * 128)
    return idx_flat, oh


def _prep(inputs):
    f32 = np.float32
    x_author = np.asarray(inputs["x_author"], f32)
    paper_tokens = np.asarray(inputs["paper_tokens"])
    paper_feat = np.asarray(inputs["paper_feat"], f32)
    edge_collab = np.asarray(inputs["edge_collab"], np.int64)
    writes_src = np.asarray(inputs["writes_src"], np.int64)
    writes_dst = np.asarray(inputs["writes_dst"], np.int64)

    src_c, dst_c = edge_collab[0], edge_collab[1]
    deg = np.bincount(dst_c, minlength=N_AUTHOR).astype(f32) + 1.0
    dinv = 1.0 / np.sqrt(deg)

    rows = _pad_author_ids(np.arange(N_AUTHOR))
    xa_gcn = np.zeros((N_A_PAD, D), dtype=BF16)       # dinv[v] * xa[v]
    xa_gcn[rows] = (x_author * dinv[:, None]).astype(BF16)
    cnt = np.bincount(writes_dst, minlength=N_PAPER).astype(f32)
    invcnt = 1.0 / np.maximum(cnt, 1.0)

    emb = np.asarray(inputs["embed_table"], f32).astype(BF16)

    # ---- GCN edge partition (per core, common layout)
    src_c_pad = _pad_author_ids(src_c)
    gcn_cores = []
    gcn_counts = []
    for c in range(N_CORES):
        m = (dst_c >= c * A_PC) & (dst_c < (c + 1) * A_PC)
        s = src_c_pad[m]
        dloc = dst_c[m] - c * A_PC
        tab = (s >= SPLIT).astype(np.int64)
        srow = np.where(tab == 0, s, s - SPLIT)
        tile = dloc >> 7
        drel = dloc & 127
        gcn_cores.append((srow, tab, tile, drel))
        cnts = np.zeros((A_TILES, 2), dtype=np.int64)
        np.add.at(cnts, (tile, tab), 1)
        gcn_counts.append(cnts)
    gcn_layout = _common_layout(gcn_counts, A_TILES, 2, GCN_GROUP)

    # ---- SAGE edge partition: table id = AllGather stripe
    ws_pad = _pad_author_ids(writes_src)
    ws_block = ws_pad // A_PC_PAD
    ws_local = ws_pad % A_PC_PAD
    ws_tile = ws_local >> 7
    ws_stripe = np.searchsorted(SLICE_T0[1:], ws_tile, side="right")
    stripe_rows = np.array([n * 128 for n in SLICE_TILES])
    off_rows = SLICE_T0[:-1] * 128
    ws_row = ws_block * stripe_rows[ws_stripe] + (ws_local - off_rows[ws_stripe])
    sage_cores = []
    sage_counts = []
    for c in range(N_CORES):
        m = (writes_dst >= c * P_PC) & (writes_dst < (c + 1) * P_PC)
        dloc = writes_dst[m] - c * P_PC
        tile = dloc >> 7
        drel = dloc & 127
        sage_cores.append((ws_row[m], ws_stripe[m], tile, drel))
        cnts = np.zeros((P_TILES, N_SLICES), dtype=np.int64)
        np.add.at(cnts, (tile, ws_stripe[m]), 1)
        sage_counts.append(cnts)
    sage_layout = _common_layout(sage_counts, P_TILES, N_SLICES, SAGE_GROUP)

    # ---- weights
    def brow(name, n):
        return np.asarray(inputs[name], f32).reshape(1, n).astype(BF16)

    Wp = np.asarray(inputs["Wp"], f32)
    wp_sb = Wp.reshape(6, 128, D_BERT).transpose(1, 0, 2).reshape(128, 6 * D_BERT)
    Wr0 = np.asarray(inputs["sage_Wr0"], f32)
    wr0_sb = Wr0[:768].reshape(6, 128, 128).transpose(1, 0, 2).reshape(128, 6 * 128)
    wr0f = Wr0[768:780]

    consts = dict(
        wp=wp_sb.astype(BF16),
        bp=brow("bp", D_BERT),
        w0=np.asarray(inputs["gcn_W0"], f32).astype(BF16),
        b0=brow("gcn_b0", D),
        wl0=np.asarray(inputs["sage_Wl0"], f32).astype(BF16),
        wr0=wr0_sb.astype(BF16),
        wr0f=wr0f.astype(BF16),
        sb0=brow("sage_b0", D),
        wl1=np.asarray(inputs["sage_Wl1"], f32).astype(BF16),
        wr1=np.asarray(inputs["sage_Wr1"], f32).astype(BF16),
        sb1=brow("sage_b1", D),
        linw=np.asarray(inputs["lin_W"], f32).astype(BF16),
        linb=brow("lin_b", OUT_D),
        ones=np.ones((1, 512), dtype=BF16),
        iden=np.eye(128, dtype=BF16),
    )

    in_maps = []
    for c in range(N_CORES):
        a0 = c * A_PC
        dv = np.zeros(A_PC_PAD, f32)
        dv[:A_PC] = dinv[a0:a0 + A_PC]
        dinvrep = np.tile(dv[None, :].astype(BF16), (128, 1))
        selfT = np.zeros((D, A_PC_PAD), dtype=BF16)
        selfT[:, :A_PC] = (x_author[a0:a0 + A_PC] *
                           (dv[:A_PC] ** 2)[:, None]).T.astype(BF16)
        cmb_shard = np.zeros((A_PC_PAD, 2 * D), dtype=BF16)
        cmb_shard[:A_PC, :D] = x_author[a0:a0 + A_PC].astype(BF16)

        p0 = c * P_PC
        iv = np.zeros(P_PC_PAD, f32)
        iv[:P_PC] = invcnt[p0:p0 + P_PC]
        invcntrep = np.tile(iv[None, :].astype(BF16), (128, 1))

        cls = np.zeros(P_PC_PAD, dtype=np.int64)
        cls[:P_PC] = paper_tokens[p0:p0 + P_PC, 0]
        featT = np.zeros((FEAT, P_PC_PAD), dtype=BF16)
        featT[:, :P_PC] = paper_feat[p0:p0 + P_PC].T.astype(BF16)

        gidx, goh = _fill_core(gcn_layout, *gcn_cores[c])
        sidx, soh = _fill_core(sage_layout, *sage_cores[c])

        m = dict(
            emb=emb,
            xa_gcn=xa_gcn,
            cmb_shard=cmb_shard,
            clsidx=_wrap_idx(cls),
            featT=featT,
            dinvrep=dinvrep,
            selfT=selfT,
            invcntrep=invcntrep,
            gcn_idx=_wrap_idx(gidx),
            gcn_oh=goh,
            sage_idx=_wrap_idx(sidx),
            sage_oh=soh,
            **consts,
        )
        in_maps.append(m)

    return in_maps, (gcn_layout, sage_layout)


def _build(gcn_layout, sage_layout):
    import concourse.bacc as bacc
    import concourse.mybir as mybir
    from concourse.tile import TileContext

    dt = mybir.dt
    AF = mybir.ActivationFunctionType
    ALU = mybir.AluOpType

    GCN_CH = gcn_layout["total_chunks"]
    SAGE_CH = sage_layout["total_chunks"]

    nc = bacc.Bacc("TRN2", target_bir_lowering=False, debug=False,
                   num_devices=N_CORES, num_swdge_queues=N_QUEUES)

    def din(name, shape, dtype=dt.bfloat16):
        return nc.dram_tensor(name, list(shape), dtype, kind="ExternalInput").ap()

    emb = din("emb", (VOCAB, D_BERT))
    xa_gcn = din("xa_gcn", (N_A_PAD, D))
    cmb_shard = din("cmb_shard", (A_PC_PAD, 2 * D))
    clsidx = din("clsidx", (128, P_PC_PAD // 16), dt.int16)
    featT = din("featT", (FEAT, P_PC_PAD))
    dinvrep = din("dinvrep", (128, A_PC_PAD))
    selfT = din("selfT", (D, A_PC_PAD))
    invcntrep = din("invcntrep", (128, P_PC_PAD))
    gcn_idx = din("gcn_idx", (128, GCN_CH * 8), dt.int16)
    gcn_oh = din("gcn_oh", (128, GCN_CH * 128), dt.float8e4)
    sage_idx = din("sage_idx", (128, SAGE_CH * 8), dt.int16)
    sage_oh = din("sage_oh", (128, SAGE_CH * 128), dt.float8e4)
    wp = din("wp", (128, 6 * D_BERT))
    bp = din("bp", (1, D_BERT))
    w0 = din("w0", (D, D))
    b0 = din("b0", (1, D))
    wl0 = din("wl0", (D, D))
    wr0 = din("wr0", (128, 6 * 128))
    wr0f = din("wr0f", (FEAT, D))
    sb0 = din("sb0", (1, D))
    wl1 = din("wl1", (D, D))
    wr1 = din("wr1", (D, D))
    sb1 = din("sb1", (1, D))
    linw = din("linw", (D, OUT_D))
    linb = din("linb", (1, OUT_D))
    ones = din("ones", (1, 512))
    iden = din("iden", (128, 128))
    out = nc.dram_tensor("out", [P_PC_PAD, OUT_D], dt.float32,
                         kind="ExternalOutput").ap()

    with TileContext(nc) as tc:
        with (
            tc.tile_pool(name="const", bufs=1) as constp,
            tc.tile_pool(name="sticky", bufs=1) as stickyp,
            tc.tile_pool(name="cls", bufs=2) as clsp,
            tc.tile_pool(name="msg", bufs=14) as msgp,
            tc.tile_pool(name="ohp", bufs=14) as ohp,
            tc.tile_pool(name="work", bufs=4) as workp,
            tc.tile_pool(name="psum", bufs=2, space="PSUM") as psump,
            tc.tile_pool(name="psagg", bufs=4, space="PSUM") as psaggp,
            tc.tile_pool(name="pspool", bufs=1, space="PSUM") as pspoolp,
            tc.tile_pool(name="dram", bufs=1, space="DRAM") as dramp,
        ):
            def load_const(ap_dram, name):
                t = constp.tile(list(ap_dram.shape), ap_dram.dtype, name=name)
                nc.sync.dma_start(out=t, in_=ap_dram)
                return t

            w0_sb = load_const(w0, "w0_sb")
            b0_sb = load_const(b0, "b0_sb")
            wl0_sb = load_const(wl0, "wl0_sb")
            wr0_sb = load_const(wr0, "wr0_sb")
            wr0f_sb = load_const(wr0f, "wr0f_sb")
            sb0_sb = load_const(sb0, "sb0_sb")
            wl1_sb = load_const(wl1, "wl1_sb")
            wr1_sb = load_const(wr1, "wr1_sb")
            sb1_sb = load_const(sb1, "sb1_sb")
            linw_sb = load_const(linw, "linw_sb")
            linb_sb = load_const(linb, "linb_sb")
            ones_sb = load_const(ones, "ones_sb")
            iden_sb = load_const(iden, "iden_sb")
            wp_sb = load_const(wp, "wp_sb")
            bp_sb = load_const(bp, "bp_sb")
            featT_sb = load_const(featT, "featT_sb")
            clsidx_sb = load_const(clsidx, "clsidx_sb")
            dinvrep_sb = load_const(dinvrep, "dinvrep_sb")
            selfT_sb = load_const(selfT, "selfT_sb")
            invcntrep_sb = load_const(invcntrep, "invcntrep_sb")
            gcn_idx_sb = load_const(gcn_idx, "gcn_idx_sb")
            sage_idx_sb = load_const(sage_idx, "sage_idx_sb")

            xpwr = stickyp.tile([128, P_PC_PAD], dt.bfloat16, name="xpwr")
            p1T = stickyp.tile([128, P_PC_PAD], dt.bfloat16, name="p1T")

            cmb_full = [
                dramp.tile([N_CORES * SLICE_TILES[s] * 128, 2 * D],
                           dt.bfloat16, addr_space="Shared",
                           name=f"cmb_full{s}")
                for s in range(N_SLICES)
            ]
            # device-owned shard buffer (Tile tracks deps on pool tiles);
            # xa half copied from the host-staged input, a1 half written
            # by the GCN consume step.
            cmb_dev = dramp.tile([A_PC_PAD, 2 * D], dt.bfloat16,
                                 name="cmb_dev")
            nc.sync.dma_start(out=cmb_dev[:, 0:D], in_=cmb_shard[:, 0:D])

            # ================= pass machinery =================
            def make_pass(layout, idx_sb, oh_dram, tables, elem, pname,
                          n_tiles, group, n_psums, chunk_mm, consume):
                calls = layout["calls"]
                tile_runs = layout["tile_runs"]
                chunk_loc = {}
                for k, (tb, c0, nchk) in enumerate(calls):
                    for i in range(nchk):
                        chunk_loc[c0 + i] = (k, i)
                msg_tiles = {}
                oh_tiles = {}

                def ensure_call(k):
                    tb, c0, nchk = calls[k]
                    mt = msgp.tile([128, CALL_CHUNKS * elem], dt.bfloat16,
                                   tag="msg", name=f"msg_{pname}_{k}")
                    nc.gpsimd.dma_gather(
                        mt[:, :nchk * elem].rearrange(
                            "p (c e) -> p c e", e=elem),
                        tables[tb], idx_sb[:, c0 * 8:(c0 + nchk) * 8],
                        nchk * 128, nchk * 128, elem)
                    ot = ohp.tile([128, CALL_CHUNKS * 128], dt.float8e4,
                                  tag="oh", name=f"oh_{pname}_{k}")
                    nc.scalar.dma_start(
                        out=ot[:, :nchk * 128],
                        in_=oh_dram[:, c0 * 128:(c0 + nchk) * 128])
                    msg_tiles[k] = mt
                    oh_tiles[k] = ot

                issued = [0]

                def issue_upto(k):
                    while issued[0] <= min(k + 6, len(calls) - 1):
                        ensure_call(issued[0])
                        issued[0] += 1

                g0 = 0
                while g0 < n_tiles:
                    g1 = min(g0 + group, n_tiles)
                    last_chunk = max(
                        r0 + nr - 1
                        for t in range(g0, g1) for (r0, nr) in tile_runs[t])
                    issue_upto(chunk_loc[last_chunk][0])
                    for t in range(g0, g1):
                        pss = [psaggp.tile([128, 128], dt.float32, tag="agg",
                                           name=f"agg_{pname}_{t}_{j}")
                               for j in range(n_psums)]
                        runs = tile_runs[t]
                        n_run_chunks = sum(nr for _, nr in runs)
                        ci = 0
                        for (r0, nr) in runs:
                            for i in range(nr):
                                k, off = chunk_loc[r0 + i]
                                chunk_mm(pss, msg_tiles[k], oh_tiles[k], off,
                                         first=(ci == 0),
                                         last=(ci == n_run_chunks - 1))
                                ci += 1
                        consume(t, pss)
                    g0 = g1

            # ================= phase 1: GCN =================
            def gcn_chunk_mm(pss, mt, ot, off, first, last):
                nc.tensor.matmul(
                    pss[0],
                    lhsT=mt[:, off * 128:(off + 1) * 128],
                    rhs=ot[:, off * 128:(off + 1) * 128],
                    start=first, stop=last)

            def gcn_consume(t, pss):
                tmp = workp.tile([128, 128], dt.bfloat16, tag="tmp",
                                 name="tmp")
                nc.vector.tensor_tensor(
                    out=tmp, in0=pss[0],
                    in1=dinvrep_sb[:, t * 128:(t + 1) * 128], op=ALU.mult)
                aggsb = workp.tile([128, 128], dt.bfloat16, tag="aggsb",
                                   name="aggsb")
                nc.vector.tensor_tensor(
                    out=aggsb, in0=tmp,
                    in1=selfT_sb[:, t * 128:(t + 1) * 128], op=ALU.add)
                lp = psump.tile([128, 128], dt.float32, tag="layer",
                                name="lp_gcn")
                nc.tensor.matmul(lp, lhsT=aggsb, rhs=w0_sb,
                                 start=True, stop=False)
                nc.tensor.matmul(lp, lhsT=ones_sb[:, :128], rhs=b0_sb,
                                 start=False, stop=True)
                a1sb = workp.tile([128, 128], dt.bfloat16, tag="a1sb",
                                  name="a1sb")
                nc.scalar.activation(a1sb, lp, AF.Relu)
                nc.sync.dma_start(
                    out=cmb_dev[t * 128:(t + 1) * 128, D:2 * D], in_=a1sb)

            make_pass(gcn_layout, gcn_idx_sb, gcn_oh,
                      [xa_gcn[0:SPLIT, :], xa_gcn[SPLIT:N_A_PAD, :]],
                      128, "gcn", A_TILES, GCN_GROUP, 1,
                      gcn_chunk_mm, gcn_consume)

            # ========= phase 2: sliced AllGather of [xa | a1] =========
            for s in range(N_SLICES):
                r0 = int(SLICE_T0[s]) * 128
                r1 = int(SLICE_T0[s + 1]) * 128
                nc.gpsimd.collective_compute(
                    "AllGather", ALU.bypass,
                    replica_groups=[list(range(N_CORES))],
                    ins=[cmb_dev[r0:r1, :].opt()],
                    outs=[cmb_full[s].opt()],
                )

            # ================= phase 3: pooler -> xpwr =================
            blks = []
            b0_ = 0
            while b0_ < P_PC_PAD:
                bw = min(BLK, P_PC_PAD - b0_)
                blks.append((b0_, bw))
                b0_ += bw
            for h, (hb, bw) in enumerate(blks):
                clsT = clsp.tile([128, 6 * BLK], dt.bfloat16, tag="clsT",
                                 name=f"clsT_{h}")
                nc.gpsimd.dma_gather(
                    clsT[:, :6 * bw].rearrange("p (c e) -> p c e", e=bw),
                    emb, clsidx_sb[:, hb // 16:(hb + bw) // 16],
                    bw, bw, D_BERT, transpose=True)
                xp_ps = pspoolp.tile([128, BLK], dt.float32, tag="xp",
                                     name=f"xp_ps{h}")
                for c in range(6):
                    po_ps = pspoolp.tile([128, BLK], dt.float32, tag="po",
                                         name=f"po_ps{h}_{c}")
                    for k in range(6):
                        nc.tensor.matmul(
                            po_ps[:, :bw],
                            lhsT=wp_sb[:, k * D_BERT + c * 128:
                                       k * D_BERT + (c + 1) * 128],
                            rhs=clsT[:, k * bw:(k + 1) * bw],
                            start=(k == 0), stop=False)
                    nc.tensor.matmul(
                        po_ps[:, :bw],
                        lhsT=bp_sb[:, c * 128:(c + 1) * 128],
                        rhs=ones_sb[:, :bw],
                        start=False, stop=True)
                    poT = clsp.tile([128, BLK], dt.bfloat16, tag="poT",
                                    name=f"poT{h}_{c}")
                    nc.scalar.activation(poT[:, :bw], po_ps[:, :bw], AF.Tanh)
                    nc.tensor.matmul(
                        xp_ps[:, :bw],
                        lhsT=wr0_sb[:, c * 128:(c + 1) * 128],
                        rhs=poT[:, :bw],
                        start=(c == 0), stop=False)
                    if c == 5:
                        nc.tensor.matmul(
                            xp_ps[:, :bw], lhsT=wr0f_sb,
                            rhs=featT_sb[:, hb:hb + bw],
                            start=False, stop=False)
                        nc.tensor.matmul(
                            xp_ps[:, :bw], lhsT=sb0_sb,
                            rhs=ones_sb[:, :bw],
                            start=False, stop=True)
                nc.vector.tensor_copy(xpwr[:, hb:hb + bw], xp_ps[:, :bw])

            # ================= phase 4: fused SAGE =================
            def sage_chunk_mm(pss, mt, ot, off, first, last):
                nc.tensor.matmul(
                    pss[0],
                    lhsT=mt[:, off * 256:off * 256 + 128],
                    rhs=ot[:, off * 128:(off + 1) * 128],
                    start=first, stop=last)
                nc.tensor.matmul(
                    pss[1],
                    lhsT=mt[:, off * 256 + 128:(off + 1) * 256],
                    rhs=ot[:, off * 128:(off + 1) * 128],
                    start=first, stop=last)

            def sage_consume(t, pss):
                mean0 = workp.tile([128, 128], dt.bfloat16, tag="mean0",
                                   name="mean0")
                nc.vector.tensor_tensor(
                    out=mean0, in0=pss[0],
                    in1=invcntrep_sb[:, t * 128:(t + 1) * 128], op=ALU.mult)
                mean1 = workp.tile([128, 128], dt.bfloat16, tag="mean1",
                                   name="mean1")
                nc.vector.tensor_tensor(
                    out=mean1, in0=pss[1],
                    in1=invcntrep_sb[:, t * 128:(t + 1) * 128], op=ALU.mult)
                pp = psump.tile([128, 128], dt.float32, tag="layer",
                                name="pp_s0")
                nc.tensor.matmul(pp, lhsT=wl0_sb, rhs=mean0,
                                 start=True, stop=False)
                nc.tensor.matmul(pp, lhsT=iden_sb,
                                 rhs=xpwr[:, t * 128:(t + 1) * 128],
                                 start=False, stop=True)
                nc.scalar.activation(p1T[:, t * 128:(t + 1) * 128], pp,
                                     AF.Relu)
                pq = psump.tile([128, 128], dt.float32, tag="layer",
                                name="pp_s1")
                nc.tensor.matmul(pq, lhsT=wl1_sb, rhs=mean1,
                                 start=True, stop=False)
                nc.tensor.matmul(pq, lhsT=wr1_sb,
                                 rhs=p1T[:, t * 128:(t + 1) * 128],
                                 start=False, stop=False)
                nc.tensor.matmul(pq, lhsT=sb1_sb, rhs=ones_sb[:, :128],
                                 start=False, stop=True)
                p2sb = workp.tile([128, 128], dt.bfloat16, tag="p2sb",
                                  name="p2sb")
                nc.scalar.activation(p2sb, pq, AF.Relu)
                hp_t = psump.tile([128, 128], dt.float32, tag="layer",
                                  name="hp")
                hp = hp_t[:, :OUT_D]
                nc.tensor.matmul(hp, lhsT=p2sb, rhs=linw_sb,
                                 start=True, stop=False)
                nc.tensor.matmul(hp, lhsT=ones_sb[:, :128], rhs=linb_sb,
                                 start=False, stop=True)
                outsb = workp.tile([128, OUT_D], dt.float32, tag="outsb",
                                   name="outsb")
                nc.vector.tensor_copy(outsb, hp)
                nc.sync.dma_start(out=out[t * 128:(t + 1) * 128, :],
                                  in_=outsb)

            make_pass(sage_layout, sage_idx_sb, sage_oh,
                      [cf[:, :] for cf in cmb_full],
                      256, "sage", P_TILES, SAGE_GROUP, 2,
                      sage_chunk_mm, sage_consume)

    # Align each SWDGE gather's queue with its Tile DMASW semaphore lane:
    # lane completions must be FIFO for Tile's cumulative sem targets, and
    # per-queue rings complete FIFO, so queue = lane % N_QUEUES keeps every
    # lane single-queue while spreading desc-gen over 4 Q7 core pairs.
    for b in nc.m.functions[0].blocks:
        for inst in b.instructions:
            if isinstance(inst, mybir.InstDMAGatherAnt):
                proc = getattr(inst, "bass_scheduled_proc", None)
                if proc is not None and 11 <= proc <= 18:
                    inst.queue_num = (proc - 11) % N_QUEUES

    nc.compile()
    return nc


def kernel(**inputs):
    from concourse import bass_utils

    in_maps, (gcn_layout, sage_layout) = _prep(inputs)

    key = (gcn_layout["total_chunks"], sage_layout["total_chunks"])
    if key in _CACHE:
        nc = _CACHE[key]
    else:
        nc = _build(gcn_layout, sage_layout)
        _CACHE[key] = nc

    kw = {}
    if TRACE:
        import os
        import shutil
        shutil.rmtree("/tmp/neff_trace", ignore_errors=True)
        os.makedirs("/tmp/neff_trace", exist_ok=True)
        kw = dict(tmpdir="/tmp/neff_trace")
    res = bass_utils.run_bass_kernel_spmd(
        nc, in_maps, core_ids=list(range(N_CORES)), trace=TRACE, **kw)
    LAST["exec_time_ns"] = res.exec_time_ns
    LAST["results"] = res

    pieces = [res.results[c]["out"][:P_PC] for c in range(N_CORES)]
    return np.concatenate(pieces, axis=0).astype(np.float32)
